# revision 1
# baseline (speedup 1.0000x reference)
"""Linear self-attention (elu+1 feature map) Trainium2 kernel.

Problem: B=4, S=4096, D=1024, H=16, HD=64.
  q = elu1(x @ Wq.T + bq); k = elu1(x @ Wk.T + bk); v = x @ Wv.T + bv
  kv_h = k_h^T v_h; ksum_h = sum_t k_h; z = 1/(q.ksum + eps)
  out = (q_h @ kv_h) * z; y = out @ Wo.T + bo

Sharding: token-parallel. Core c handles batch c//2, sequence half c%2
(2048 tokens). kv/ksum are partial sums over local tokens, AllReduced
across the 2-core group sharing a batch, then every core finishes its
own tokens through attention + output projection. bo is added on host.

x is shipped pre-transposed (chunk-major x^T) so the projection
matmuls need no on-device transpose of x.
"""

import numpy as np
from contextlib import ExitStack

import concourse.bass as bass
import concourse.tile as tile
from concourse import bacc, mybir
from concourse.bass_utils import run_bass_kernel_spmd
from concourse.tile_rust import add_dep_helper

B, S, D, H, HD = 4, 4096, 1024, 16, 64
N_CORES = 8
TOK = (B * S) // N_CORES      # 2048 tokens per core
NT = TOK // 128               # 16 token tiles per core
F32 = mybir.dt.float32
F32R = mybir.dt.float32r
BF16 = mybir.dt.bfloat16
EPS = 1e-6

# dtype for the big matmuls (projections, attention, output projection).
# float32r = relaxed-precision fp32 (4-byte storage), bfloat16 = 2-byte.
MM_DT = F32R

TRACE = False            # set by test harness for profiling
LAST_RESULT = None       # BassKernelResults of last run
DEBUG = False            # dump tile-0 intermediates to extra outputs

_PROGRAMS = {}


def _emit(nc, has_bias, mm_dt):
    AF = mybir.ActivationFunctionType
    ALU = mybir.AluOpType
    esz = mybir.dt.size(mm_dt)

    # x^T, chunk-major per token tile: [p, t*1024 + c*128 + j] =
    #   x[t*128 + j, c*128 + p]
    xst = nc.dram_tensor("xst", [128, NT * 1024], mm_dt, kind="ExternalInput").ap()
    wkd = nc.dram_tensor("wkt", [128, 8 * D], mm_dt, kind="ExternalInput").ap()
    wvd = nc.dram_tensor("wvt", [128, 8 * D], mm_dt, kind="ExternalInput").ap()
    wqd = nc.dram_tensor("wqt", [128, 8 * D], mm_dt, kind="ExternalInput").ap()
    wod = nc.dram_tensor("wot", [128, 8 * D], mm_dt, kind="ExternalInput").ap()
    identd = nc.dram_tensor("ident", [128, 128], F32, kind="ExternalInput").ap()
    biasd = nc.dram_tensor("biases", [1, 4096], F32, kind="ExternalInput").ap()
    y_d = nc.dram_tensor("y", [TOK, D], F32, kind="ExternalOutput").ap()
    qspill = nc.dram_tensor("qspill", [128, NT * 1024], mm_dt).ap()
    cc_in = nc.dram_tensor("cc_in", [64, 1040], F32).ap()
    cc_out = nc.dram_tensor("cc_out", [64, 1040], F32).ap()

    dbg = {}
    if DEBUG:
        for name, shape in [("dbg_k", [128, 1024]), ("dbg_v", [128, 1024]),
                            ("dbg_q", [128, 1024]),
                            ("dbg_kv", [64, 1040]), ("dbg_bd", [128, 2048]),
                            ("dbg_od", [128, 2048]), ("dbg_osb", [128, 1024])]:
            dbg[name] = nc.dram_tensor(name, shape, F32, kind="ExternalOutput").ap()

    with tile.TileContext(nc) as tc, ExitStack() as top:
        wpool = top.enter_context(tc.tile_pool(name="w", bufs=3))
        cpool = top.enter_context(tc.tile_pool(name="const", bufs=1))
        ident = cpool.tile([128, 128], F32, tag="ident")
        nc.sync.dma_start(ident[:], identd)
        if has_bias:
            ones_row_st = cpool.tile([1, 512], F32, tag="ones_row_st")
            nc.vector.memset(ones_row_st[:], 1.0)
            ones_row = cpool.tile([1, 128], mm_dt, tag="ones_row")
            nc.vector.tensor_copy(ones_row[:], ones_row_st[0:1, 0:128])
            ones_row512 = cpool.tile([1, 512], mm_dt, tag="ones_row512")
            nc.vector.tensor_copy(ones_row512[:], ones_row_st[:])
            bias_st = cpool.tile([1, 3072], F32, tag="bias_st")
            nc.sync.dma_start(bias_st[:], biasd[0:1, 0:3072])
            bias_sb = cpool.tile([1, 3072], mm_dt, tag="bias")
            nc.vector.tensor_copy(bias_sb[:], bias_st[:])

        def load_weight(dram_ap):
            # weights stay off the sync queue (x tiles live there); chunked
            # across SWDGE + scalar-HWDGE so the first projection matmuls
            # only wait for the first ~1 MiB
            wt = wpool.tile([128, 8 * D], mm_dt, tag="w")
            for c in range(8):
                nc.gpsimd.dma_start(wt[:, c * D:(c + 1) * D],
                                    dram_ap[:, c * D:(c + 1) * D])
            return wt

        wk_t = load_weight(wkd)
        wv_t = load_weight(wvd)
        wq_t = load_weight(wqd)

        kvstack = ExitStack()
        kvpool = kvstack.enter_context(tc.tile_pool(name="kvp", bufs=1, space="PSUM"))
        # augmented kv accumulator: head h at cols h*128, cols 0:64 = kv_h,
        # col 64 = ksum_h (from the ones column appended to v)
        kv_ps = kvpool.tile([64, 2048], F32, tag="kv")

        # ---------------- Pass 1: q/k/v projections, kv + ksum ----------------
        GT = 4                     # token tiles per x^T group (N=512 q matmuls)
        NG = NT // GT
        with ExitStack() as p1:
            xtpool = p1.enter_context(tc.tile_pool(name="xt", bufs=2))
            kqv_pool = p1.enter_context(tc.tile_pool(name="kqv", bufs=2))
            mepool = p1.enter_context(tc.tile_pool(name="me", bufs=2))
            qtpool = p1.enter_context(tc.tile_pool(name="qt", bufs=1))
            projp = p1.enter_context(tc.tile_pool(name="projp", bufs=4, space="PSUM"))

            def add_bias(ps, boff, g):
                if has_bias:
                    nc.tensor.matmul(
                        ps[:, g * 512:(g + 1) * 512],
                        ones_row[0:1, 0:128],
                        bias_sb[0:1, boff + g * 512: boff + g * 512 + 512],
                        start=False, stop=True,
                    )

            def elu1_half(dst_half, ps_half):
                # elu(x)+1 = exp(min(x,0)) + max(x,0), on a [128,512] half
                me = mepool.tile([128, 512], F32, tag="me")
                nc.vector.tensor_scalar_min(me[:], ps_half, 0.0)
                nc.scalar.activation(me[:], me[:], AF.Exp)
                nc.vector.scalar_tensor_tensor(
                    dst_half, ps_half, 0.0, me[:], ALU.max, ALU.add)

            # software pipeline over groups of GT=4 token tiles sharing one
            # x^T buffer:
            #   A1(t) = k/v projections for one tile (+ elu/copy on DVE)
            #   B(t)  = augmented kv matmuls for tile t (k^T @ [v|1])
            #   A2(g) = q^T computed directly (weights stationary, N=512)
            #           + elu in transposed layout, spilled to DRAM
            # B(t-1) is emitted between A1 stages so the in-order PE always
            # has matmul work while DVE/ACT run elu.
            st = {}

            def stage_a1(t, xtg):
                tt = t % GT
                ksb = kqv_pool.tile([128, 1024], F32, tag="k")
                vsb = kqv_pool.tile([128, 1040], F32, tag="v")
                khalves = []
                for g in range(2):
                    kh = projp.tile([128, 512], F32, tag="proj", name=f"kps{t}_{g}")
                    for c in range(8):
                        nc.tensor.matmul(
                            kh[:], xtg[:, c * 512 + tt * 128: c * 512 + tt * 128 + 128],
                            wk_t[:, c * D + g * 512: c * D + g * 512 + 512],
                            start=(c == 0), stop=(c == 7 and not has_bias))
                    add_bias(kh, 1024, g)
                    khalves.append(kh)
                for g in range(2):
                    vh = projp.tile([128, 512], F32, tag="proj", name=f"vps{t}_{g}")
                    for c in range(8):
                        nc.tensor.matmul(
                            vh[:], xtg[:, c * 512 + tt * 128: c * 512 + tt * 128 + 128],
                            wv_t[:, c * D + g * 512: c * D + g * 512 + 512],
                            start=(c == 0), stop=(c == 7 and not has_bias))
                    add_bias(vh, 2048, g)
                    # strided copy into the [v | 1] augmented layout
                    nc.vector.tensor_copy(
                        vsb[:, g * 520: g * 520 + 520]
                        .rearrange("p (h e) -> p h e", e=65)[:, :, 0:64],
                        vh[:].rearrange("p (h e) -> p h e", e=64))
                nc.vector.memset(
                    vsb[:].rearrange("p (h e) -> p h e", e=65)[:, :, 64:65], 1.0)
                for g in range(2):
                    elu1_half(ksb[:, g * 512:(g + 1) * 512], khalves[g][:])
                st[t] = (ksb, vsb)

            def stage_b(t):
                ksb, vsb = st.pop(t)
                for h in range(16):
                    # NOTE: start=True clears has_written for the whole PSUM
                    # bank (4 heads), so only the first matmul per bank sets it
                    nc.tensor.matmul(
                        kv_ps[0:64, h * 128: h * 128 + 65],
                        ksb[:, h * 64:(h + 1) * 64],
                        vsb[:, h * 65: h * 65 + 65],
                        start=(t == 0 and h % 4 == 0), stop=(t == NT - 1),
                    )
                if DEBUG and t == 0:
                    nc.sync.dma_start(dbg["dbg_k"][:], ksb[:])
                    nc.sync.dma_start(dbg["dbg_v"][:], vsb[:, 0:1024])

            def stage_a2(g, xtg, after=None):
                qtsb = qtpool.tile([128, 4096], mm_dt, tag="qt")
                for dqc in range(8):
                    qh = projp.tile([128, 512], F32, tag="proj", name=f"qps{g}_{dqc}")
                    for dc in range(8):
                        mmi = nc.tensor.matmul(
                            qh[:],
                            wq_t[:, dc * D + dqc * 128: dc * D + dqc * 128 + 128],
                            xtg[:, dc * 512:(dc + 1) * 512],
                            start=(dc == 0), stop=(dc == 7 and not has_bias))
                        if after is not None and dc == 0:
                            # keep the scheduler from hoisting this work
                            # before the collective send it should hide
                            add_dep_helper(mmi.ins, after.ins, sync=True,
                                           reason="defer q^T behind cc send")
                    if has_bias:
                        # q^T bias: bq along partitions = rank-1 with ones row
                        nc.tensor.matmul(
                            qh[:],
                            bias_sb[0:1, dqc * 128: dqc * 128 + 128],
                            ones_row512[0:1, 0:512],
                            start=False, stop=True)
                    elu1_half(qtsb[:, dqc * 512:(dqc + 1) * 512], qh[:])
                # spill, rearranged to per-token-tile chunk-major layout
                # (one DMA per token tile: DMA APs are limited to 3 dims)
                for tt in range(GT):
                    t = g * GT + tt
                    nc.sync.dma_start(
                        qspill[:, t * 1024:(t + 1) * 1024]
                        .rearrange("p (c j) -> p c j", j=128),
                        qtsb[:].rearrange("p (c u) -> p c u", u=512)
                        [:, :, tt * 128:(tt + 1) * 128])

            def send_kv():
                kvsb = cpool.tile([64, 1040], F32, tag="kvsb")
                kv_r = kv_ps[:].rearrange("p (h r) -> p h r", r=128)
                nc.vector.tensor_copy(
                    kvsb[:, 0:1024].rearrange("p (h e) -> p h e", e=64),
                    kv_r[:, :, 0:64])
                nc.vector.tensor_copy(
                    kvsb[:, 1024:1040].unsqueeze(2), kv_r[:, :, 64:65])
                send = nc.sync.dma_start(cc_in[:], kvsb[:])
                if DEBUG:
                    nc.sync.dma_start(dbg["dbg_kv"][:], kvsb[:])
                nc.gpsimd.collective_compute(
                    "AllReduce", mybir.AluOpType.add,
                    replica_groups=[[0, 1], [2, 3], [4, 5], [6, 7]],
                    ins=[cc_in[:]], outs=[cc_out[:]],
                )
                return send

            prev = None
            for g in range(NG):
                xtg = xtpool.tile([128, GT * 1024], mm_dt, tag="xt")
                if g == 0:
                    # split so the first tile's matmuls start after ~0.5 MiB
                    for tt in range(GT):
                        nc.sync.dma_start(
                            xtg[:].rearrange("p (c u) -> p c u", u=512)
                            [:, :, tt * 128:(tt + 1) * 128],
                            xst[:, 0:GT * 1024]
                            .rearrange("p (c u) -> p c u", u=512)
                            [:, :, tt * 128:(tt + 1) * 128])
                else:
                    nc.sync.dma_start(xtg[:], xst[:, g * GT * 1024:(g + 1) * GT * 1024])
                for tt in range(GT):
                    t = g * GT + tt
                    stage_a1(t, xtg)
                    if prev is not None:
                        stage_b(prev)
                    prev = t
                if g < NG - 1:
                    stage_a2(g, xtg)
                else:
                    # last group: finish kv, launch the AllReduce, THEN do
                    # the q^T stage so its ~15us of matmuls hide CC latency
                    stage_b(prev)
                    prev = None
                    send = send_kv()
                    stage_a2(g, xtg, after=send)
            if prev is not None:
                stage_b(prev)

        kvstack.close()

        # ---------------- Pass 2: attention + output projection ----------------
        with ExitStack() as p2:
            bdpool = p2.enter_context(tc.tile_pool(name="bd", bufs=1))
            qtip = p2.enter_context(tc.tile_pool(name="qti", bufs=3))
            out_pool = p2.enter_context(tc.tile_pool(name="osb", bufs=2))
            outT_pool = p2.enter_context(tc.tile_pool(name="otb", bufs=2))
            y_pool = p2.enter_context(tc.tile_pool(name="ysb", bufs=2))
            zpool = p2.enter_context(tc.tile_pool(name="z", bufs=2))
            odp = p2.enter_context(tc.tile_pool(name="odp", bufs=2, space="PSUM"))
            tpp2 = p2.enter_context(tc.tile_pool(name="tpp2", bufs=2, space="PSUM"))
            ypp = p2.enter_context(tc.tile_pool(name="ypp", bufs=1, space="PSUM"))

            wo_t = load_weight(wod)

            # block-diagonal [kv | ksum] matrix: chunk c (heads 2c, 2c+1):
            # rows 0:64 = head 2c (d), rows 64:128 = head 2c+1
            # cols c*256+[0:64] = kv_2c, [64:128] = kv_2c+1, 128/129 = ksums
            assert esz == 4, "direct-DMA BD build assumes 4-byte mm_dt"
            bd = bdpool.tile([128, 2048], mm_dt, tag="bd")
            nc.vector.memset(bd[:].bitcast(F32), 0.0)
            cc_kv = (cc_out[0:64, 0:1024].bitcast(mm_dt)
                     .rearrange("p (j i e) -> p i j e", i=2, e=64))
            bd_lo = bd[0:64, :].rearrange("p (c r) -> p c r", r=256)
            bd_hi = bd[64:128, :].rearrange("p (c r) -> p c r", r=256)
            nc.sync.dma_start(bd_lo[:, :, 0:64], cc_kv[:, 0, :, :])
            nc.sync.dma_start(bd_hi[:, :, 64:128], cc_kv[:, 1, :, :])
            cc_ks = (cc_out[0:64, 1024:1040].bitcast(mm_dt)
                     .rearrange("p (j i) -> p j i", i=2))
            nc.sync.dma_start(bd_lo[:, :, 128:129], cc_ks[:, :, 0:1])
            nc.sync.dma_start(bd_hi[:, :, 129:130], cc_ks[:, :, 1:2])
            if DEBUG:
                nc.sync.dma_start(dbg["dbg_bd"][:], bd[:].bitcast(F32))

            # software pipeline: stage A(t) = qti DMA + attention matmuls,
            # stage B(t) = normalize/scale + transpose + output projection.
            # A(t+1) is emitted before B(t) so the PE has matmul work while
            # the vector engine normalizes tile t.
            state = {}

            def stage_a(t):
                qti = qtip.tile([128, 1024], mm_dt, tag="qti")
                nc.sync.dma_start(qti[:], qspill[:, t * 1024:(t + 1) * 1024])
                ods = [odp.tile([128, 1024], F32, tag="od", name=f"od{t}_{i}")
                       for i in range(2)]
                zden = zpool.tile([128, 16], F32, tag="zden")
                for half in range(2):
                    od = ods[half]
                    for cc in range(4):
                        c = half * 4 + cc
                        nc.tensor.matmul(
                            od[:, cc * 256:(cc + 1) * 256],
                            qti[:, c * 128:(c + 1) * 128],
                            bd[:, c * 256:(c + 1) * 256],
                            start=True, stop=True,
                        )
                    od_r = od[:].rearrange("p (c r) -> p c r", r=256)
                    nc.vector.tensor_copy(
                        zden[:, half * 8:(half + 1) * 8]
                        .rearrange("p (c i) -> p c i", i=2),
                        od_r[:, :, 128:130])
                state[t] = (ods, zden)

            def stage_b(t):
                ods, zden = state.pop(t)
                zinv = zpool.tile([128, 16], F32, tag="zinv")
                nc.vector.tensor_scalar_add(zden[:], zden[:], EPS)
                nc.vector.reciprocal(zinv[:], zden[:])
                osb = out_pool.tile([128, 1024], F32, tag="osb")
                for half in range(2):
                    od_r = ods[half][:].rearrange("p (c r) -> p c r", r=256)
                    zb = (zinv[:, half * 8:(half + 1) * 8]
                          .rearrange("p (c i) -> p c i", i=2)
                          .unsqueeze(3).broadcast_to((128, 4, 2, 64)))
                    nc.vector.tensor_mul(
                        osb[:, half * 512:(half + 1) * 512]
                        .rearrange("p (c i e) -> p c i e", c=4, i=2),
                        od_r[:, :, 0:128].rearrange("p c (i e) -> p c i e", i=2),
                        zb,
                    )

                otb = outT_pool.tile([128, 1024], mm_dt, tag="otb")
                for c in range(8):
                    tp2 = tpp2.tile([128, 128], F32, tag="tp2")
                    nc.tensor.transpose(tp2[:], osb[:, c * 128:(c + 1) * 128], ident[:])
                    nc.vector.tensor_copy(otb[:, c * 128:(c + 1) * 128], tp2[:])

                yps = ypp.tile([128, 1024], F32, tag="y")
                for c in range(8):
                    lhs = otb[:, c * 128:(c + 1) * 128]
                    for g in range(2):
                        nc.tensor.matmul(
                            yps[:, g * 512:(g + 1) * 512], lhs,
                            wo_t[:, c * D + g * 512: c * D + g * 512 + 512],
                            start=(c == 0), stop=(c == 7),
                        )
                ysb = y_pool.tile([128, 1024], F32, tag="ysb")
                nc.vector.tensor_copy(ysb[:], yps[:])
                nc.sync.dma_start(y_d[t * 128:(t + 1) * 128, :], ysb[:])

                if DEBUG and t == 0:
                    for half in range(2):
                        odsb = y_pool.tile([128, 1024], F32, tag="odsb",
                                           name=f"odsb{half}")
                        nc.vector.tensor_copy(odsb[:], ods[half][:])
                        nc.sync.dma_start(
                            dbg["dbg_od"][:, half * 1024:(half + 1) * 1024], odsb[:])
                    nc.sync.dma_start(dbg["dbg_osb"][:], osb[:])

            stage_a(0)
            for t in range(1, NT):
                stage_a(t)
                stage_b(t - 1)
            stage_b(NT - 1)


def _get_program(has_bias):
    key = (has_bias, MM_DT)
    if key not in _PROGRAMS:
        nc = bacc.Bacc("TRN2", target_bir_lowering=False, debug=False,
                       num_devices=N_CORES)
        _emit(nc, has_bias, MM_DT)
        nc.compile()
        _PROGRAMS[key] = nc
    return _PROGRAMS[key]


def _to_mm_np(a):
    """Convert fp32 array to the numpy dtype matching MM_DT."""
    if MM_DT == BF16:
        import ml_dtypes
        return np.ascontiguousarray(a.astype(ml_dtypes.bfloat16))
    return np.ascontiguousarray(a)


def _pack_rhs(w):
    # W [out,in] -> W.T chunk-major rhs layout [128, 8*1024]:
    # [p, c*1024 + n] = W.T[c*128+p, n]
    return _to_mm_np(
        w.T.reshape(8, 128, D).transpose(1, 0, 2).reshape(128, 8 * D))


def _pack_xt(xs):
    # xs [TOK, D] -> x^T group-major: [p, g*4096 + c*512 + u] = xs[g*512+u, c*128+p]
    ng = NT // 4
    return _to_mm_np(
        xs.T.reshape(8, 128, ng, 512).transpose(1, 2, 0, 3).reshape(128, NT * 1024))


def kernel(x, Wq, bq, Wk, bk, Wv, bv, Wo, bo):
    global LAST_RESULT
    x = np.asarray(x, dtype=np.float32)
    Wq, Wk, Wv, Wo = (np.asarray(w, dtype=np.float32) for w in (Wq, Wk, Wv, Wo))
    bq, bk, bv, bo = (np.asarray(b, dtype=np.float32) for b in (bq, bk, bv, bo))

    has_bias = bool(np.any(bq) or np.any(bk) or np.any(bv))
    nc = _get_program(has_bias)
    shared = {
        "wkt": _pack_rhs(Wk),
        "wvt": _pack_rhs(Wv),
        "wqt": _pack_rhs(Wq),
        "wot": _pack_rhs(Wo),
        "ident": np.eye(128, dtype=np.float32),
        "biases": np.concatenate([bq, bk, bv, bo]).reshape(1, 4096),
    }
    in_maps = []
    for c in range(N_CORES):
        b = c // 2
        h = c % 2
        m = dict(shared)
        m["xst"] = _pack_xt(x[b, h * TOK:(h + 1) * TOK, :])
        in_maps.append(m)

    res = run_bass_kernel_spmd(nc, in_maps, list(range(N_CORES)), trace=TRACE)
    LAST_RESULT = res

    y = np.empty((B, S, D), dtype=np.float32)
    for c in range(N_CORES):
        b = c // 2
        h = c % 2
        y[b, h * TOK:(h + 1) * TOK, :] = res.results[c]["y"]
    y += bo
    return y



# revision 3
# speedup vs baseline: 1.1541x; 1.1541x over previous
"""Linear self-attention (elu+1 feature map) Trainium2 kernel — bf16.

Problem: B=4, S=4096, D=1024, H=16, HD=64.
  q = elu1(x @ Wq.T + bq); k = elu1(x @ Wk.T + bk); v = x @ Wv.T + bv
  kv_h = k_h^T v_h; ksum_h = sum_t k_h; z = 1/(q.ksum + eps)
  out = (q_h @ kv_h) * z; y = out @ Wo.T + bo

Sharding: token-parallel. Core c handles batch c//2, sequence half c%2
(2048 tokens). kv/ksum are partial sums over local tokens, AllReduced
(bf16) across the 2-core group sharing a batch, then every core
finishes its own tokens through attention + output projection. bo is
added on host.

All PE-facing operands are bf16: fp32r moving operands run at ~2
cycles/row on TRN2 hardware and fp32 kv matmuls ran as LOW_HIGH pairs;
bf16 runs at 1 cycle/row. PSUM accumulation stays fp32. q^T stays
resident in SBUF (no DRAM spill). kv matmuls batch 2 heads per
instruction. Two q^T groups are deferred until after the kv AllReduce
send so the collective is hidden under matmul work.
"""

import numpy as np
from contextlib import ExitStack

import concourse.bass as bass
import concourse.tile as tile
from concourse import bacc, mybir
from concourse.bass_utils import run_bass_kernel_spmd
from concourse.tile_rust import add_dep_helper

B, S, D, H, HD = 4, 4096, 1024, 16, 64
N_CORES = 8
TOK = (B * S) // N_CORES      # 2048 tokens per core
NT = TOK // 128               # 16 token tiles per core
GT = 4                        # token tiles per x^T group
NG = NT // GT
DEFER = 2                     # q^T groups emitted after the kv CC send
F32 = mybir.dt.float32
BF16 = mybir.dt.bfloat16
EPS = 1e-6

MM_DT = BF16

TRACE = False            # set by test harness for profiling
LAST_RESULT = None       # BassKernelResults of last run

_PROGRAMS = {}


def _emit(nc, has_bias, mm_dt):
    AF = mybir.ActivationFunctionType
    ALU = mybir.AluOpType

    # x^T, chunk-major per token tile within each group:
    # [p, g*4096 + c*512 + u] = x[g*512 + u, c*128 + p]
    xst = nc.dram_tensor("xst", [128, NT * 1024], mm_dt, kind="ExternalInput").ap()
    wkd = nc.dram_tensor("wkt", [128, 8 * D], mm_dt, kind="ExternalInput").ap()
    wvd = nc.dram_tensor("wvt", [128, 8 * D], mm_dt, kind="ExternalInput").ap()
    wqd = nc.dram_tensor("wqt", [128, 8 * D], mm_dt, kind="ExternalInput").ap()
    wod = nc.dram_tensor("wot", [128, 8 * D], mm_dt, kind="ExternalInput").ap()
    identd = nc.dram_tensor("ident", [128, 128], mm_dt, kind="ExternalInput").ap()
    biasd = nc.dram_tensor("biases", [1, 4096], F32, kind="ExternalInput").ap()
    y_d = nc.dram_tensor("y", [TOK, D], F32, kind="ExternalOutput").ap()
    cc_in = nc.dram_tensor("cc_in", [64, 1040], mm_dt).ap()
    cc_out = nc.dram_tensor("cc_out", [64, 1040], mm_dt).ap()

    with tile.TileContext(nc) as tc, ExitStack() as top:
        wpool = top.enter_context(tc.tile_pool(name="w", bufs=4))
        cpool = top.enter_context(tc.tile_pool(name="const", bufs=1))
        qtpool = top.enter_context(tc.tile_pool(name="qt", bufs=1))
        ospool = top.enter_context(tc.tile_pool(name="os", bufs=1))
        identm = cpool.tile([128, 128], mm_dt, tag="ident")
        nc.sync.dma_start(identm[:], identd)
        if has_bias:
            ones_row_st = cpool.tile([1, 512], F32, tag="ones_row_st")
            nc.vector.memset(ones_row_st[:], 1.0)
            ones_row = cpool.tile([1, 128], mm_dt, tag="ones_row")
            nc.vector.tensor_copy(ones_row[:], ones_row_st[0:1, 0:128])
            ones_row512 = cpool.tile([1, 512], mm_dt, tag="ones_row512")
            nc.vector.tensor_copy(ones_row512[:], ones_row_st[:])
            bias_st = cpool.tile([1, 3072], F32, tag="bias_st")
            nc.sync.dma_start(bias_st[:], biasd[0:1, 0:3072])
            bias_sb = cpool.tile([1, 3072], mm_dt, tag="bias")
            nc.vector.tensor_copy(bias_sb[:], bias_st[:])

        def load_weight(dram_ap):
            # weights stay off the sync queue (x tiles live there); chunked
            # so the first projection matmuls only wait for the first ~256 KiB
            wt = wpool.tile([128, 8 * D], mm_dt, tag="w")
            for c in range(8):
                nc.gpsimd.dma_start(wt[:, c * D:(c + 1) * D],
                                    dram_ap[:, c * D:(c + 1) * D])
            return wt

        wk_t = load_weight(wkd)
        wv_t = load_weight(wvd)
        wq_t = load_weight(wqd)
        wo_t = load_weight(wod)

        kvstack = ExitStack()
        kvpool = kvstack.enter_context(tc.tile_pool(name="kvp", bufs=1, space="PSUM"))
        # 2-head-batched kv accumulator: block r (heads 2r, 2r+1) at cols
        # r*256: rows 0:64 x cols 0:65 = [kv_2r | ksum_2r], rows 64:128 x
        # cols 65:130 = [kv_2r+1 | ksum_2r+1]; the other corners are unused
        kv_ps = kvpool.tile([128, 2048], F32, tag="kv")

        qts = {}

        # ---------------- Pass 1: q/k/v projections, kv + ksum ----------------
        with ExitStack() as p1:
            xtpool = p1.enter_context(tc.tile_pool(name="xt", bufs=3))
            kqv_pool = p1.enter_context(tc.tile_pool(name="kqv", bufs=2))
            mepool = p1.enter_context(tc.tile_pool(name="me", bufs=2))
            projp = p1.enter_context(tc.tile_pool(name="projp", bufs=4, space="PSUM"))

            def add_bias(ps, boff, g):
                if has_bias:
                    nc.tensor.matmul(
                        ps[:],
                        ones_row[0:1, 0:128],
                        bias_sb[0:1, boff + g * 512: boff + g * 512 + 512],
                        start=False, stop=True,
                    )

            def elu1_half(dst_half, ps_half):
                # elu(x)+1 = exp(min(x,0)) + max(x,0), on a [128,512] half
                me = mepool.tile([128, 512], F32, tag="me")
                nc.vector.tensor_scalar_min(me[:], ps_half, 0.0)
                nc.scalar.activation(me[:], me[:], AF.Exp)
                nc.vector.scalar_tensor_tensor(
                    dst_half, ps_half, 0.0, me[:], ALU.max, ALU.add)

            # software pipeline over groups of GT=4 token tiles sharing one
            # x^T buffer:
            #   A1(t) = k/v projections for one tile (+ elu/copy on DVE)
            #   B(t)  = 2-head-batched kv matmuls for tile t (k^T @ [v|1])
            #   A2(g) = q^T computed directly (weights stationary, N=512)
            #           + elu in transposed layout, kept resident in SBUF
            # B(t-1) is emitted between A1 stages so the in-order PE always
            # has matmul work while DVE/ACT run elu.
            st = {}

            def stage_a1(t, xtg):
                tt = t % GT
                ksb = kqv_pool.tile([128, 1024], mm_dt, tag="k")
                vsb = kqv_pool.tile([128, 1040], mm_dt, tag="v")
                khalves = []
                for g in range(2):
                    kh = projp.tile([128, 512], F32, tag="proj", name=f"kps{t}_{g}")
                    for c in range(8):
                        nc.tensor.matmul(
                            kh[:], xtg[:, c * 512 + tt * 128: c * 512 + tt * 128 + 128],
                            wk_t[:, c * D + g * 512: c * D + g * 512 + 512],
                            start=(c == 0), stop=(c == 7 and not has_bias))
                    add_bias(kh, 1024, g)
                    khalves.append(kh)
                for g in range(2):
                    vh = projp.tile([128, 512], F32, tag="proj", name=f"vps{t}_{g}")
                    for c in range(8):
                        nc.tensor.matmul(
                            vh[:], xtg[:, c * 512 + tt * 128: c * 512 + tt * 128 + 128],
                            wv_t[:, c * D + g * 512: c * D + g * 512 + 512],
                            start=(c == 0), stop=(c == 7 and not has_bias))
                    add_bias(vh, 2048, g)
                    # strided copy into the [v | 1] augmented layout
                    nc.vector.tensor_copy(
                        vsb[:, g * 520: g * 520 + 520]
                        .rearrange("p (h e) -> p h e", e=65)[:, :, 0:64],
                        vh[:].rearrange("p (h e) -> p h e", e=64))
                nc.vector.memset(
                    vsb[:].rearrange("p (h e) -> p h e", e=65)[:, :, 64:65], 1.0)
                for g in range(2):
                    elu1_half(ksb[:, g * 512:(g + 1) * 512], khalves[g][:])
                st[t] = (ksb, vsb)

            def stage_b(t):
                ksb, vsb = st.pop(t)
                for r in range(8):
                    # NOTE: start=True clears has_written for the whole PSUM
                    # bank (2 blocks), so only the even block per bank sets it
                    nc.tensor.matmul(
                        kv_ps[:, r * 256: r * 256 + 130],
                        ksb[:, r * 128:(r + 1) * 128],
                        vsb[:, r * 130: r * 130 + 130],
                        start=(t == 0 and r % 2 == 0), stop=(t == NT - 1),
                    )

            def stage_a2(g, xtg, after=None):
                qtsb = qtpool.tile([128, 4096], mm_dt, tag=f"qt{g}")
                qts[g] = qtsb
                for dqc in range(8):
                    qh = projp.tile([128, 512], F32, tag="proj", name=f"qps{g}_{dqc}")
                    for dc in range(8):
                        mmi = nc.tensor.matmul(
                            qh[:],
                            wq_t[:, dc * D + dqc * 128: dc * D + dqc * 128 + 128],
                            xtg[:, dc * 512:(dc + 1) * 512],
                            start=(dc == 0), stop=(dc == 7 and not has_bias))
                        if after is not None and dqc == 0 and dc == 0:
                            # keep the scheduler from hoisting this work
                            # before the collective send it should hide
                            add_dep_helper(mmi.ins, after.ins, sync=True,
                                           reason="defer q^T behind cc send")
                    if has_bias:
                        # q^T bias: bq along partitions = rank-1 with ones row
                        nc.tensor.matmul(
                            qh[:],
                            bias_sb[0:1, dqc * 128: dqc * 128 + 128],
                            ones_row512[0:1, 0:512],
                            start=False, stop=True)
                    elu1_half(qtsb[:, dqc * 512:(dqc + 1) * 512], qh[:])

            def send_kv():
                # PSUM f32 -> bf16 staging (same partitions), then small
                # SBUF->SBUF DMAs assemble the [64, 1040] cc payload
                # (odd heads need a partition shift, hence DMA not DVE)
                kvb16 = cpool.tile([128, 2048], mm_dt, tag="kvb16")
                nc.vector.tensor_copy(
                    kvb16[:].rearrange("p (r w) -> p r w", w=256)[:, :, 0:130],
                    kv_ps[:].rearrange("p (r w) -> p r w", w=256)[:, :, 0:130])
                kvsb = cpool.tile([64, 1040], mm_dt, tag="kvsb")
                kv_even = kvb16[0:64, :].rearrange("p (r w) -> p r w", w=256)
                kv_odd = kvb16[64:128, :].rearrange("p (r w) -> p r w", w=256)
                dst_kv = kvsb[:, 0:1024].rearrange("p (r i e) -> p r i e", i=2, e=64)
                dst_ks = kvsb[:, 1024:1040].rearrange("p (r i) -> p r i", i=2)
                nc.sync.dma_start(dst_kv[:, :, 0, :], kv_even[:, :, 0:64])
                nc.sync.dma_start(dst_kv[:, :, 1, :], kv_odd[:, :, 65:129])
                nc.sync.dma_start(dst_ks[:, :, 0:1], kv_even[:, :, 64:65])
                nc.sync.dma_start(dst_ks[:, :, 1:2], kv_odd[:, :, 129:130])
                send = nc.sync.dma_start(cc_in[:], kvsb[:])
                nc.gpsimd.collective_compute(
                    "AllReduce", mybir.AluOpType.add,
                    replica_groups=[[0, 1], [2, 3], [4, 5], [6, 7]],
                    ins=[cc_in[:]], outs=[cc_out[:]],
                )
                return send

            prev = None
            deferred = []
            for g in range(NG):
                xtg = xtpool.tile([128, GT * 1024], mm_dt, tag="xt")
                if g == 0:
                    # split so the first tile's matmuls start after ~256 KiB
                    for tt in range(GT):
                        nc.sync.dma_start(
                            xtg[:].rearrange("p (c u) -> p c u", u=512)
                            [:, :, tt * 128:(tt + 1) * 128],
                            xst[:, 0:GT * 1024]
                            .rearrange("p (c u) -> p c u", u=512)
                            [:, :, tt * 128:(tt + 1) * 128])
                else:
                    nc.sync.dma_start(xtg[:], xst[:, g * GT * 1024:(g + 1) * GT * 1024])
                for tt in range(GT):
                    t = g * GT + tt
                    stage_a1(t, xtg)
                    if prev is not None:
                        stage_b(prev)
                    prev = t
                if g < NG - DEFER:
                    stage_a2(g, xtg)
                else:
                    deferred.append((g, xtg))
            # finish kv, launch the AllReduce, THEN the deferred q^T groups
            # (~27us of matmuls) hide the collective latency
            stage_b(prev)
            send = send_kv()
            for i, (g, xtg) in enumerate(deferred):
                stage_a2(g, xtg, after=send if i == 0 else None)

        kvstack.close()

        # ---------------- Pass 2a: attention + normalize -> osb (SBUF) --------
        osbs = {}
        with ExitStack() as p2a:
            bdpool = p2a.enter_context(tc.tile_pool(name="bd", bufs=1))
            zpool = p2a.enter_context(tc.tile_pool(name="z", bufs=2))
            odp = p2a.enter_context(tc.tile_pool(name="odp", bufs=2, space="PSUM"))

            # block-diagonal [kv | ksum] matrix: chunk c (heads 2c, 2c+1):
            # rows 0:64 = head 2c (d), rows 64:128 = head 2c+1
            # cols c*256+[0:64] = kv_2c, [64:128] = kv_2c+1, 128/129 = ksums
            bd = bdpool.tile([128, 2048], mm_dt, tag="bd")
            nc.vector.memset(bd[:].bitcast(F32), 0.0)
            cc_kv = (cc_out[0:64, 0:1024]
                     .rearrange("p (j i e) -> p i j e", i=2, e=64))
            bd_lo = bd[0:64, :].rearrange("p (c r) -> p c r", r=256)
            bd_hi = bd[64:128, :].rearrange("p (c r) -> p c r", r=256)
            nc.sync.dma_start(bd_lo[:, :, 0:64], cc_kv[:, 0, :, :])
            nc.sync.dma_start(bd_hi[:, :, 64:128], cc_kv[:, 1, :, :])
            cc_ks = (cc_out[0:64, 1024:1040].rearrange("p (j i) -> p j i", i=2))
            nc.sync.dma_start(bd_lo[:, :, 128:129], cc_ks[:, :, 0:1])
            nc.sync.dma_start(bd_hi[:, :, 129:130], cc_ks[:, :, 1:2])

            # attention for tile t; od double-buffered across tiles (all 8
            # PSUM banks) so attn(t+1) runs while DVE normalizes tile t
            for t in range(NT):
                g, tt = t // GT, t % GT
                qtsb = qts[g]
                ods = [odp.tile([128, 1024], F32, tag=f"od{i}", name=f"od{t}_{i}")
                       for i in range(2)]
                zden = zpool.tile([128, 16], F32, tag="zden")
                for half in range(2):
                    od = ods[half]
                    for cc in range(4):
                        c = half * 4 + cc
                        nc.tensor.matmul(
                            od[:, cc * 256:(cc + 1) * 256],
                            qtsb[:, c * 512 + tt * 128: c * 512 + tt * 128 + 128],
                            bd[:, c * 256:(c + 1) * 256],
                            start=True, stop=True,
                        )
                    od_r = od[:].rearrange("p (c r) -> p c r", r=256)
                    nc.vector.tensor_copy(
                        zden[:, half * 8:(half + 1) * 8]
                        .rearrange("p (c i) -> p c i", i=2),
                        od_r[:, :, 128:130])
                zinv = zpool.tile([128, 16], F32, tag="zinv")
                nc.vector.tensor_scalar_add(zden[:], zden[:], EPS)
                nc.vector.reciprocal(zinv[:], zden[:])
                osb = ospool.tile([128, 1024], mm_dt, tag=f"osb{t}")
                osbs[t] = osb
                for half in range(2):
                    od_r = ods[half][:].rearrange("p (c r) -> p c r", r=256)
                    zb = (zinv[:, half * 8:(half + 1) * 8]
                          .rearrange("p (c i) -> p c i", i=2)
                          .unsqueeze(3).broadcast_to((128, 4, 2, 64)))
                    nc.vector.tensor_mul(
                        osb[:, half * 512:(half + 1) * 512]
                        .rearrange("p (c i e) -> p c i e", c=4, i=2),
                        od_r[:, :, 0:128].rearrange("p c (i e) -> p c i e", i=2),
                        zb,
                    )

        # ---------------- Pass 2b: transpose + output projection --------------
        with ExitStack() as p2b:
            outT_pool = p2b.enter_context(tc.tile_pool(name="otb", bufs=2))
            y_pool = p2b.enter_context(tc.tile_pool(name="ysb", bufs=2))
            tpp2 = p2b.enter_context(tc.tile_pool(name="tpp2", bufs=2, space="PSUM"))
            ypp = p2b.enter_context(tc.tile_pool(name="ypp", bufs=2, space="PSUM"))

            for t in range(NT):
                osb = osbs.pop(t)
                otb = outT_pool.tile([128, 1024], mm_dt, tag="otb")
                for c in range(8):
                    tp2 = tpp2.tile([128, 128], mm_dt, tag="tp2")
                    nc.tensor.transpose(tp2[:], osb[:, c * 128:(c + 1) * 128],
                                        identm[:])
                    nc.vector.tensor_copy(otb[:, c * 128:(c + 1) * 128], tp2[:])

                yps = ypp.tile([128, 1024], F32, tag="y")
                for c in range(8):
                    lhs = otb[:, c * 128:(c + 1) * 128]
                    for g in range(2):
                        nc.tensor.matmul(
                            yps[:, g * 512:(g + 1) * 512], lhs,
                            wo_t[:, c * D + g * 512: c * D + g * 512 + 512],
                            start=(c == 0), stop=(c == 7),
                        )
                ysb = y_pool.tile([128, 1024], F32, tag="ysb")
                nc.vector.tensor_copy(ysb[:], yps[:])
                nc.sync.dma_start(y_d[t * 128:(t + 1) * 128, :], ysb[:])


def _get_program(has_bias):
    key = (has_bias, MM_DT)
    if key not in _PROGRAMS:
        nc = bacc.Bacc("TRN2", target_bir_lowering=False, debug=False,
                       num_devices=N_CORES)
        _emit(nc, has_bias, MM_DT)
        nc.compile()
        _PROGRAMS[key] = nc
    return _PROGRAMS[key]


def _to_mm_np(a):
    """Convert fp32 array to the numpy dtype matching MM_DT."""
    if MM_DT == BF16:
        import ml_dtypes
        return np.ascontiguousarray(a.astype(ml_dtypes.bfloat16))
    return np.ascontiguousarray(a)


def _pack_rhs(w):
    # W [out,in] -> W.T chunk-major rhs layout [128, 8*1024]:
    # [p, c*1024 + n] = W.T[c*128+p, n]
    return _to_mm_np(
        w.T.reshape(8, 128, D).transpose(1, 0, 2).reshape(128, 8 * D))


def _pack_xt(xs):
    # xs [TOK, D] -> x^T group-major: [p, g*4096 + c*512 + u] = xs[g*512+u, c*128+p]
    ng = NT // 4
    return _to_mm_np(
        xs.T.reshape(8, 128, ng, 512).transpose(1, 2, 0, 3).reshape(128, NT * 1024))


def kernel(x, Wq, bq, Wk, bk, Wv, bv, Wo, bo):
    global LAST_RESULT
    x = np.asarray(x, dtype=np.float32)
    Wq, Wk, Wv, Wo = (np.asarray(w, dtype=np.float32) for w in (Wq, Wk, Wv, Wo))
    bq, bk, bv, bo = (np.asarray(b, dtype=np.float32) for b in (bq, bk, bv, bo))

    has_bias = bool(np.any(bq) or np.any(bk) or np.any(bv))
    nc = _get_program(has_bias)
    shared = {
        "wkt": _pack_rhs(Wk),
        "wvt": _pack_rhs(Wv),
        "wqt": _pack_rhs(Wq),
        "wot": _pack_rhs(Wo),
        "ident": _to_mm_np(np.eye(128, dtype=np.float32)),
        "biases": np.concatenate([bq, bk, bv, bo]).reshape(1, 4096),
    }
    in_maps = []
    for c in range(N_CORES):
        b = c // 2
        h = c % 2
        m = dict(shared)
        m["xst"] = _pack_xt(x[b, h * TOK:(h + 1) * TOK, :])
        in_maps.append(m)

    res = run_bass_kernel_spmd(nc, in_maps, list(range(N_CORES)), trace=TRACE)
    LAST_RESULT = res

    y = np.empty((B, S, D), dtype=np.float32)
    for c in range(N_CORES):
        b = c // 2
        h = c % 2
        y[b, h * TOK:(h + 1) * TOK, :] = res.results[c]["y"]
    y += bo
    return y


# revision 14
# speedup vs baseline: 1.1964x; 1.0367x over previous
"""Linear self-attention (elu+1 feature map) Trainium2 kernel — bf16.

Problem: B=4, S=4096, D=1024, H=16, HD=64.
  q = elu1(x @ Wq.T + bq); k = elu1(x @ Wk.T + bk); v = x @ Wv.T + bv
  kv_h = k_h^T v_h; ksum_h = sum_t k_h; z = 1/(q.ksum + eps)
  out = (q_h @ kv_h) * z; y = out @ Wo.T + bo

Sharding: token-parallel. Core c handles batch c//2, sequence half c%2
(2048 tokens). kv/ksum are partial sums over local tokens, AllReduced
(bf16) across the 2-core group sharing a batch, then every core
finishes its own tokens through attention + output projection. bo is
added on host.

All PE-facing operands are bf16: fp32r moving operands run at ~2
cycles/row on TRN2 hardware and fp32 kv matmuls ran as LOW_HIGH pairs;
bf16 runs at 1 cycle/row. PSUM accumulation stays fp32. q^T stays
resident in SBUF (no DRAM spill). kv matmuls batch 2 heads per
instruction. Two q^T groups are deferred until after the kv AllReduce
send so the collective is hidden under matmul work.
"""

import numpy as np
from contextlib import ExitStack

import concourse.bass as bass
import concourse.tile as tile
from concourse import bacc, mybir
from concourse.bass_utils import run_bass_kernel_spmd

B, S, D, H, HD = 4, 4096, 1024, 16, 64
N_CORES = 8
TOK = (B * S) // N_CORES      # 2048 tokens per core
NT = TOK // 128               # 16 token tiles per core
GT = 4                        # token tiles per x^T group
NG = NT // GT
DEFER = 2                     # q^T groups emitted after the kv CC send
F32 = mybir.dt.float32
BF16 = mybir.dt.bfloat16
EPS = 1e-6

MM_DT = BF16

TRACE = False            # set by test harness for profiling
LAST_RESULT = None       # BassKernelResults of last run

_PROGRAMS = {}


def _emit(nc, has_bias, mm_dt):
    AF = mybir.ActivationFunctionType
    ALU = mybir.AluOpType

    # x^T, chunk-major per token tile within each group:
    # [p, g*4096 + c*512 + u] = x[g*512 + u, c*128 + p]
    xst = nc.dram_tensor("xst", [128, NT * 1024], mm_dt, kind="ExternalInput").ap()
    wkd = nc.dram_tensor("wkt", [128, 8 * D], mm_dt, kind="ExternalInput").ap()
    wvd = nc.dram_tensor("wvt", [128, 8 * D], mm_dt, kind="ExternalInput").ap()
    wqd = nc.dram_tensor("wqt", [128, 8 * D], mm_dt, kind="ExternalInput").ap()
    wod = nc.dram_tensor("wot", [128, 8 * D], mm_dt, kind="ExternalInput").ap()
    identd = nc.dram_tensor("ident", [128, 128], mm_dt, kind="ExternalInput").ap()
    biasd = nc.dram_tensor("biases", [1, 4096], F32, kind="ExternalInput").ap()
    y_d = nc.dram_tensor("y", [TOK, D], F32, kind="ExternalOutput").ap()
    cc_in = nc.dram_tensor("cc_in", [64, 1040], mm_dt).ap()
    cc_out = nc.dram_tensor("cc_out", [64, 1040], mm_dt).ap()

    with tile.TileContext(nc) as tc, ExitStack() as top:
        wpool = top.enter_context(tc.tile_pool(name="w", bufs=4))
        cpool = top.enter_context(tc.tile_pool(name="const", bufs=1))
        qtpool = top.enter_context(tc.tile_pool(name="qt", bufs=1))
        ospool = top.enter_context(tc.tile_pool(name="os", bufs=1))
        identm = cpool.tile([128, 128], mm_dt, tag="ident")
        nc.sync.dma_start(identm[:], identd)
        if has_bias:
            ones_row_st = cpool.tile([1, 512], F32, tag="ones_row_st")
            nc.vector.memset(ones_row_st[:], 1.0)
            ones_row = cpool.tile([1, 128], mm_dt, tag="ones_row")
            nc.vector.tensor_copy(ones_row[:], ones_row_st[0:1, 0:128])
            ones_row512 = cpool.tile([1, 512], mm_dt, tag="ones_row512")
            nc.vector.tensor_copy(ones_row512[:], ones_row_st[:])
            bias_st = cpool.tile([1, 3072], F32, tag="bias_st")
            nc.sync.dma_start(bias_st[:], biasd[0:1, 0:3072])
            bias_sb = cpool.tile([1, 3072], mm_dt, tag="bias")
            nc.vector.tensor_copy(bias_sb[:], bias_st[:])

        def load_weight(dram_ap):
            # weights stay off the sync queue (x tiles live there); chunked
            # so the first projection matmuls only wait for the first ~256 KiB
            wt = wpool.tile([128, 8 * D], mm_dt, tag="w")
            for c in range(8):
                nc.gpsimd.dma_start(wt[:, c * D:(c + 1) * D],
                                    dram_ap[:, c * D:(c + 1) * D])
            return wt

        wk_t = load_weight(wkd)
        wv_t = load_weight(wvd)
        wq_t = load_weight(wqd)
        wo_t = load_weight(wod)

        kvstack = ExitStack()
        kvpool = kvstack.enter_context(tc.tile_pool(name="kvp", bufs=1, space="PSUM"))
        # 2-head-batched kv accumulator: block r (heads 2r, 2r+1) at cols
        # r*256: rows 0:64 x cols 0:65 = [kv_2r | ksum_2r], rows 64:128 x
        # cols 65:130 = [kv_2r+1 | ksum_2r+1]; the other corners are unused
        kv_ps = kvpool.tile([128, 2048], F32, tag="kv")

        qts = {}

        # ---------------- Pass 1: q/k/v projections, kv + ksum ----------------
        with ExitStack() as p1:
            xtpool = p1.enter_context(tc.tile_pool(name="xt", bufs=3))
            kqv_pool = p1.enter_context(tc.tile_pool(name="kqv", bufs=2))
            mepool = p1.enter_context(tc.tile_pool(name="me", bufs=2))
            projp = p1.enter_context(tc.tile_pool(name="projp", bufs=4, space="PSUM"))

            def add_bias(ps, boff, g):
                if has_bias:
                    nc.tensor.matmul(
                        ps[:],
                        ones_row[0:1, 0:128],
                        bias_sb[0:1, boff + g * 512: boff + g * 512 + 512],
                        start=False, stop=True,
                    )

            def elu1_half(dst_half, ps_half):
                # elu(x)+1 = exp(min(x,0)) + max(x,0), on a [128,512] half
                me = mepool.tile([128, 512], F32, tag="me")
                nc.vector.tensor_scalar_min(me[:], ps_half, 0.0)
                nc.scalar.activation(me[:], me[:], AF.Exp)
                nc.vector.scalar_tensor_tensor(
                    dst_half, ps_half, 0.0, me[:], ALU.max, ALU.add)

            # software pipeline over groups of GT=4 token tiles sharing one
            # x^T buffer:
            #   A1(t) = k/v projections for one tile (+ elu/copy on DVE)
            #   B(t)  = 2-head-batched kv matmuls for tile t (k^T @ [v|1])
            #   A2(g) = q^T computed directly (weights stationary, N=512)
            #           + elu in transposed layout, kept resident in SBUF
            # B(t-1) is emitted between A1 stages so the in-order PE always
            # has matmul work while DVE/ACT run elu.
            st = {}

            def stage_a1(t, xtg):
                tt = t % GT
                ksb = kqv_pool.tile([128, 1024], mm_dt, tag="k")
                vsb = kqv_pool.tile([128, 1040], mm_dt, tag="v")
                khalves = []
                for g in range(2):
                    kh = projp.tile([128, 512], F32, tag="proj", name=f"kps{t}_{g}")
                    for c in range(8):
                        nc.tensor.matmul(
                            kh[:], xtg[:, c * 512 + tt * 128: c * 512 + tt * 128 + 128],
                            wk_t[:, c * D + g * 512: c * D + g * 512 + 512],
                            start=(c == 0), stop=(c == 7 and not has_bias))
                    add_bias(kh, 1024, g)
                    khalves.append(kh)
                for g in range(2):
                    vh = projp.tile([128, 512], F32, tag="proj", name=f"vps{t}_{g}")
                    for c in range(8):
                        nc.tensor.matmul(
                            vh[:], xtg[:, c * 512 + tt * 128: c * 512 + tt * 128 + 128],
                            wv_t[:, c * D + g * 512: c * D + g * 512 + 512],
                            start=(c == 0), stop=(c == 7 and not has_bias))
                    add_bias(vh, 2048, g)
                    # strided copy into the [v | 1] augmented layout
                    nc.vector.tensor_copy(
                        vsb[:, g * 520: g * 520 + 520]
                        .rearrange("p (h e) -> p h e", e=65)[:, :, 0:64],
                        vh[:].rearrange("p (h e) -> p h e", e=64))
                nc.vector.memset(
                    vsb[:].rearrange("p (h e) -> p h e", e=65)[:, :, 64:65], 1.0)
                for g in range(2):
                    elu1_half(ksb[:, g * 512:(g + 1) * 512], khalves[g][:])
                st[t] = (ksb, vsb)

            def stage_b(t):
                ksb, vsb = st.pop(t)
                for r in range(8):
                    # NOTE: start=True clears has_written for the whole PSUM
                    # bank (2 blocks), so only the even block per bank sets it
                    nc.tensor.matmul(
                        kv_ps[:, r * 256: r * 256 + 130],
                        ksb[:, r * 128:(r + 1) * 128],
                        vsb[:, r * 130: r * 130 + 130],
                        start=(t == 0 and r % 2 == 0), stop=(t == NT - 1),
                    )

            def stage_a2(g, xtg):
                qtsb = qtpool.tile([128, 4096], mm_dt, tag=f"qt{g}")
                qts[g] = qtsb
                for dqc in range(8):
                    qh = projp.tile([128, 512], F32, tag="proj", name=f"qps{g}_{dqc}")
                    for dc in range(8):
                        nc.tensor.matmul(
                            qh[:],
                            wq_t[:, dc * D + dqc * 128: dc * D + dqc * 128 + 128],
                            xtg[:, dc * 512:(dc + 1) * 512],
                            start=(dc == 0), stop=(dc == 7 and not has_bias))
                    if has_bias:
                        # q^T bias: bq along partitions = rank-1 with ones row
                        nc.tensor.matmul(
                            qh[:],
                            bias_sb[0:1, dqc * 128: dqc * 128 + 128],
                            ones_row512[0:1, 0:512],
                            start=False, stop=True)
                    elu1_half(qtsb[:, dqc * 512:(dqc + 1) * 512], qh[:])

            def send_kv():
                # PSUM f32 -> bf16 staging (same partitions), then small
                # SBUF->SBUF DMAs assemble the [64, 1040] cc payload
                # (odd heads need a partition shift, hence DMA not DVE)
                kvb16 = cpool.tile([128, 2048], mm_dt, tag="kvb16")
                nc.vector.tensor_copy(
                    kvb16[:].rearrange("p (r w) -> p r w", w=256)[:, :, 0:130],
                    kv_ps[:].rearrange("p (r w) -> p r w", w=256)[:, :, 0:130])
                kvsb = cpool.tile([64, 1040], mm_dt, tag="kvsb")
                kv_even = kvb16[0:64, :].rearrange("p (r w) -> p r w", w=256)
                kv_odd = kvb16[64:128, :].rearrange("p (r w) -> p r w", w=256)
                dst_kv = kvsb[:, 0:1024].rearrange("p (r i e) -> p r i e", i=2, e=64)
                dst_ks = kvsb[:, 1024:1040].rearrange("p (r i) -> p r i", i=2)
                nc.sync.dma_start(dst_kv[:, :, 0, :], kv_even[:, :, 0:64])
                nc.sync.dma_start(dst_kv[:, :, 1, :], kv_odd[:, :, 65:129])
                nc.sync.dma_start(dst_ks[:, :, 0:1], kv_even[:, :, 64:65])
                nc.sync.dma_start(dst_ks[:, :, 1:2], kv_odd[:, :, 129:130])
                nc.sync.dma_start(cc_in[:], kvsb[:])
                nc.gpsimd.collective_compute(
                    "AllReduce", mybir.AluOpType.add,
                    replica_groups=[[0, 1], [2, 3], [4, 5], [6, 7]],
                    ins=[cc_in[:]], outs=[cc_out[:]],
                )

            prev = None
            deferred = []
            for g in range(NG):
                xtg = xtpool.tile([128, GT * 1024], mm_dt, tag="xt")
                if g == 0:
                    # per-chunk contiguous DMAs (1 KiB/partition runs) so the
                    # first matmuls only wait for chunk 0 and the loads keep
                    # pace with the c-loop consumption order
                    for c in range(8):
                        nc.sync.dma_start(
                            xtg[:, c * 512:(c + 1) * 512],
                            xst[:, c * 512:(c + 1) * 512])
                else:
                    nc.sync.dma_start(xtg[:], xst[:, g * GT * 1024:(g + 1) * GT * 1024])
                for tt in range(GT):
                    t = g * GT + tt
                    stage_a1(t, xtg)
                    if prev is not None:
                        stage_b(prev)
                    prev = t
                if g < NG - DEFER:
                    stage_a2(g, xtg)
                else:
                    deferred.append((g, xtg))
            # finish kv, launch the AllReduce, THEN the deferred q^T groups
            # (~27us of matmuls) hide the collective latency; the send chain
            # (DVE copy + DMAs + CC) runs concurrently since q^T touches
            # neither the sync queue nor kv data
            stage_b(prev)
            send_kv()
            for g, xtg in deferred:
                stage_a2(g, xtg)

        kvstack.close()

        # ---------------- Pass 2a: attention + normalize -> osb (SBUF) --------
        osbs = {}
        with ExitStack() as p2a:
            bdpool = p2a.enter_context(tc.tile_pool(name="bd", bufs=1))
            zpool = p2a.enter_context(tc.tile_pool(name="z", bufs=2))
            odp = p2a.enter_context(tc.tile_pool(name="odp", bufs=2, space="PSUM"))

            # block-diagonal [kv | ksum] matrix: chunk c (heads 2c, 2c+1):
            # rows 0:64 = head 2c (d), rows 64:128 = head 2c+1
            # cols c*256+[0:64] = kv_2c, [64:128] = kv_2c+1, 128/129 = ksums
            bd = bdpool.tile([128, 2048], mm_dt, tag="bd")
            nc.vector.memset(bd[:].bitcast(F32), 0.0)
            cc_kv = (cc_out[0:64, 0:1024]
                     .rearrange("p (j i e) -> p i j e", i=2, e=64))
            bd_lo = bd[0:64, :].rearrange("p (c r) -> p c r", r=256)
            bd_hi = bd[64:128, :].rearrange("p (c r) -> p c r", r=256)
            nc.sync.dma_start(bd_lo[:, :, 0:64], cc_kv[:, 0, :, :])
            nc.sync.dma_start(bd_hi[:, :, 64:128], cc_kv[:, 1, :, :])
            cc_ks = (cc_out[0:64, 1024:1040].rearrange("p (j i) -> p j i", i=2))
            nc.sync.dma_start(bd_lo[:, :, 128:129], cc_ks[:, :, 0:1])
            nc.sync.dma_start(bd_hi[:, :, 129:130], cc_ks[:, :, 1:2])

            # attention for tile t; od double-buffered across tiles (all 8
            # PSUM banks) so attn(t+1) runs while DVE normalizes tile t
            for t in range(NT):
                g, tt = t // GT, t % GT
                qtsb = qts[g]
                ods = [odp.tile([128, 1024], F32, tag=f"od{i}", name=f"od{t}_{i}")
                       for i in range(2)]
                zden = zpool.tile([128, 16], F32, tag="zden")
                for half in range(2):
                    od = ods[half]
                    for cc in range(4):
                        c = half * 4 + cc
                        nc.tensor.matmul(
                            od[:, cc * 256:(cc + 1) * 256],
                            qtsb[:, c * 512 + tt * 128: c * 512 + tt * 128 + 128],
                            bd[:, c * 256:(c + 1) * 256],
                            start=True, stop=True,
                        )
                    od_r = od[:].rearrange("p (c r) -> p c r", r=256)
                    # scalar engine is idle here; keep the DVE free for the
                    # z-scale (gpsimd cannot access PSUM)
                    nc.scalar.activation(
                        zden[:, half * 8:(half + 1) * 8]
                        .rearrange("p (c i) -> p c i", i=2),
                        od_r[:, :, 128:130], AF.Copy)
                zinv = zpool.tile([128, 16], F32, tag="zinv")
                nc.vector.tensor_scalar_add(zden[:], zden[:], EPS)
                nc.vector.reciprocal(zinv[:], zden[:])
                osb = ospool.tile([128, 1024], mm_dt, tag=f"osb{t}")
                osbs[t] = osb
                for half in range(2):
                    od_r = ods[half][:].rearrange("p (c r) -> p c r", r=256)
                    zb = (zinv[:, half * 8:(half + 1) * 8]
                          .rearrange("p (c i) -> p c i", i=2)
                          .unsqueeze(3).broadcast_to((128, 4, 2, 64)))
                    nc.vector.tensor_mul(
                        osb[:, half * 512:(half + 1) * 512]
                        .rearrange("p (c i e) -> p c i e", c=4, i=2),
                        od_r[:, :, 0:128].rearrange("p c (i e) -> p c i e", i=2),
                        zb,
                    )

        # ---------------- Pass 2b: transpose + output projection --------------
        with ExitStack() as p2b:
            outT_pool = p2b.enter_context(tc.tile_pool(name="otb", bufs=2))
            y_pool = p2b.enter_context(tc.tile_pool(name="ysb", bufs=2))
            tpp2 = p2b.enter_context(tc.tile_pool(name="tpp2", bufs=2, space="PSUM"))
            ypp = p2b.enter_context(tc.tile_pool(name="ypp", bufs=2, space="PSUM"))

            for t in range(NT):
                osb = osbs.pop(t)
                otb = outT_pool.tile([128, 1024], mm_dt, tag="otb")
                for c in range(8):
                    tp2 = tpp2.tile([128, 128], mm_dt, tag="tp2")
                    nc.tensor.transpose(tp2[:], osb[:, c * 128:(c + 1) * 128],
                                        identm[:])
                    nc.vector.tensor_copy(otb[:, c * 128:(c + 1) * 128], tp2[:])

                yps = ypp.tile([128, 1024], F32, tag="y")
                for c in range(8):
                    lhs = otb[:, c * 128:(c + 1) * 128]
                    for g in range(2):
                        nc.tensor.matmul(
                            yps[:, g * 512:(g + 1) * 512], lhs,
                            wo_t[:, c * D + g * 512: c * D + g * 512 + 512],
                            start=(c == 0), stop=(c == 7),
                        )
                ysb = y_pool.tile([128, 1024], F32, tag="ysb")
                # scalar-engine copy keeps the DVE queue clear for the otb
                # copies the wo matmuls of the next tile wait on (gpsimd
                # cannot access PSUM)
                nc.scalar.activation(ysb[:], yps[:], AF.Copy)
                nc.sync.dma_start(y_d[t * 128:(t + 1) * 128, :], ysb[:])


def _get_program(has_bias):
    key = (has_bias, MM_DT)
    if key not in _PROGRAMS:
        nc = bacc.Bacc("TRN2", target_bir_lowering=False, debug=False,
                       num_devices=N_CORES)
        _emit(nc, has_bias, MM_DT)
        nc.compile()
        _PROGRAMS[key] = nc
    return _PROGRAMS[key]


def _to_mm_np(a):
    """Convert fp32 array to the numpy dtype matching MM_DT."""
    if MM_DT == BF16:
        import ml_dtypes
        return np.ascontiguousarray(a.astype(ml_dtypes.bfloat16))
    return np.ascontiguousarray(a)


def _pack_rhs(w):
    # W [out,in] -> W.T chunk-major rhs layout [128, 8*1024]:
    # [p, c*1024 + n] = W.T[c*128+p, n]
    return _to_mm_np(
        w.T.reshape(8, 128, D).transpose(1, 0, 2).reshape(128, 8 * D))


def _pack_xt(xs):
    # xs [TOK, D] -> x^T group-major: [p, g*4096 + c*512 + u] = xs[g*512+u, c*128+p]
    ng = NT // 4
    return _to_mm_np(
        xs.T.reshape(8, 128, ng, 512).transpose(1, 2, 0, 3).reshape(128, NT * 1024))


def kernel(x, Wq, bq, Wk, bk, Wv, bv, Wo, bo):
    global LAST_RESULT
    x = np.asarray(x, dtype=np.float32)
    Wq, Wk, Wv, Wo = (np.asarray(w, dtype=np.float32) for w in (Wq, Wk, Wv, Wo))
    bq, bk, bv, bo = (np.asarray(b, dtype=np.float32) for b in (bq, bk, bv, bo))

    has_bias = bool(np.any(bq) or np.any(bk) or np.any(bv))
    nc = _get_program(has_bias)
    shared = {
        "wkt": _pack_rhs(Wk),
        "wvt": _pack_rhs(Wv),
        "wqt": _pack_rhs(Wq),
        "wot": _pack_rhs(Wo),
        "ident": _to_mm_np(np.eye(128, dtype=np.float32)),
        "biases": np.concatenate([bq, bk, bv, bo]).reshape(1, 4096),
    }
    in_maps = []
    for c in range(N_CORES):
        b = c // 2
        h = c % 2
        m = dict(shared)
        m["xst"] = _pack_xt(x[b, h * TOK:(h + 1) * TOK, :])
        in_maps.append(m)

    res = run_bass_kernel_spmd(nc, in_maps, list(range(N_CORES)), trace=TRACE)
    LAST_RESULT = res

    y = np.empty((B, S, D), dtype=np.float32)
    for c in range(N_CORES):
        b = c // 2
        h = c % 2
        y[b, h * TOK:(h + 1) * TOK, :] = res.results[c]["y"]
    y += bo
    return y


# revision 22
# speedup vs baseline: 1.2228x; 1.0220x over previous
"""Linear self-attention (elu+1 feature map) Trainium2 kernel — bf16.

Problem: B=4, S=4096, D=1024, H=16, HD=64.
  q = elu1(x @ Wq.T + bq); k = elu1(x @ Wk.T + bk); v = x @ Wv.T + bv
  kv_h = k_h^T v_h; ksum_h = sum_t k_h; z = 1/(q.ksum + eps)
  out = (q_h @ kv_h) * z; y = out @ Wo.T + bo

Sharding: token-parallel. Core c handles batch c//2, sequence half c%2
(2048 tokens). kv/ksum are partial sums over local tokens, AllReduced
(bf16) across the 2-core group sharing a batch, then every core
finishes its own tokens through attention + output projection. bo is
added on host.

All PE-facing operands are bf16: fp32r moving operands run at ~2
cycles/row on TRN2 hardware and fp32 kv matmuls ran as LOW_HIGH pairs;
bf16 runs at 1 cycle/row. PSUM accumulation stays fp32. q^T stays
resident in SBUF (no DRAM spill). kv matmuls batch 2 heads per
instruction. Two q^T groups are deferred until after the kv AllReduce
send so the collective is hidden under matmul work.
"""

import numpy as np
from contextlib import ExitStack

import concourse.bass as bass
import concourse.tile as tile
from concourse import bacc, mybir
from concourse.bass_utils import run_bass_kernel_spmd
from concourse.tile_rust import add_dep_helper

B, S, D, H, HD = 4, 4096, 1024, 16, 64
N_CORES = 8
TOK = (B * S) // N_CORES      # 2048 tokens per core
NT = TOK // 128               # 16 token tiles per core
GT = 4                        # token tiles per x^T group
NG = NT // GT
F32 = mybir.dt.float32
BF16 = mybir.dt.bfloat16
EPS = 1e-6

MM_DT = BF16

TRACE = False            # set by test harness for profiling
LAST_RESULT = None       # BassKernelResults of last run

_PROGRAMS = {}


def _emit(nc, has_bias, mm_dt):
    AF = mybir.ActivationFunctionType
    ALU = mybir.AluOpType

    # x^T, chunk-major per token tile within each group:
    # [p, g*4096 + c*512 + u] = x[g*512 + u, c*128 + p]
    xst = nc.dram_tensor("xst", [128, NT * 1024], mm_dt, kind="ExternalInput").ap()
    wkd = nc.dram_tensor("wkt", [128, 8 * D], mm_dt, kind="ExternalInput").ap()
    wvd = nc.dram_tensor("wvt", [128, 8 * D], mm_dt, kind="ExternalInput").ap()
    wqd = nc.dram_tensor("wqt", [128, 8 * D], mm_dt, kind="ExternalInput").ap()
    wod = nc.dram_tensor("wot", [128, 8 * D], mm_dt, kind="ExternalInput").ap()
    identd = nc.dram_tensor("ident", [128, 128], mm_dt, kind="ExternalInput").ap()
    biasd = nc.dram_tensor("biases", [1, 4096], F32, kind="ExternalInput").ap()
    y_d = nc.dram_tensor("y", [TOK, D], F32, kind="ExternalOutput").ap()
    # kv collective payload keeps the PSUM block layout: block r (heads
    # 2r, 2r+1) at cols r*130; rows 0:64 x 0:65 = [kv_2r | ksum_2r],
    # rows 64:128 x 65:130 = [kv_2r+1 | ksum_2r+1] (complement is junk)
    cc_in = nc.dram_tensor("cc_in", [128, 1040], mm_dt).ap()
    cc_out = nc.dram_tensor("cc_out", [128, 1040], mm_dt).ap()

    with tile.TileContext(nc) as tc, ExitStack() as top:
        wpool = top.enter_context(tc.tile_pool(name="w", bufs=4))
        cpool = top.enter_context(tc.tile_pool(name="const", bufs=1))
        qtpool = top.enter_context(tc.tile_pool(name="qt", bufs=1))
        ospool = top.enter_context(tc.tile_pool(name="os", bufs=1))
        identm = cpool.tile([128, 128], mm_dt, tag="ident")
        nc.sync.dma_start(identm[:], identd)
        if has_bias:
            ones_row_st = cpool.tile([1, 512], F32, tag="ones_row_st")
            nc.vector.memset(ones_row_st[:], 1.0)
            ones_row = cpool.tile([1, 128], mm_dt, tag="ones_row")
            nc.vector.tensor_copy(ones_row[:], ones_row_st[0:1, 0:128])
            ones_row512 = cpool.tile([1, 512], mm_dt, tag="ones_row512")
            nc.vector.tensor_copy(ones_row512[:], ones_row_st[:])
            bias_st = cpool.tile([1, 3072], F32, tag="bias_st")
            nc.sync.dma_start(bias_st[:], biasd[0:1, 0:3072])
            bias_sb = cpool.tile([1, 3072], mm_dt, tag="bias")
            nc.vector.tensor_copy(bias_sb[:], bias_st[:])

        def load_weight(dram_ap, after=None):
            # weights stay off the sync queue (x tiles live there); chunked
            # so the first projection matmuls only wait for the first ~256 KiB.
            # `after` delays the load (sync dep on a prior instruction) so
            # late-use weights don't contend for early HBM bandwidth.
            wt = wpool.tile([128, 8 * D], mm_dt, tag="w")
            for c in range(8):
                dma = nc.gpsimd.dma_start(wt[:, c * D:(c + 1) * D],
                                          dram_ap[:, c * D:(c + 1) * D])
                if after is not None and c == 0:
                    add_dep_helper(dma.ins, after.ins, sync=True,
                                   reason="defer weight load off early HBM")
            return wt

        wk_t = load_weight(wkd)
        wv_t = load_weight(wvd)

        kvstack = ExitStack()
        kvpool = kvstack.enter_context(tc.tile_pool(name="kvp", bufs=1, space="PSUM"))
        # 2-head-batched kv accumulator: block r (heads 2r, 2r+1) at cols
        # r*256: rows 0:64 x cols 0:65 = [kv_2r | ksum_2r], rows 64:128 x
        # cols 65:130 = [kv_2r+1 | ksum_2r+1]; the other corners are unused
        kv_ps = kvpool.tile([128, 2048], F32, tag="kv")

        qts = {}

        # ---------------- Pass 1: q/k/v projections, kv + ksum ----------------
        with ExitStack() as p1:
            xtpool = p1.enter_context(tc.tile_pool(name="xt", bufs=4))
            kqv_pool = p1.enter_context(tc.tile_pool(name="kqv", bufs=2))
            mepool = p1.enter_context(tc.tile_pool(name="me", bufs=2))
            projp = p1.enter_context(tc.tile_pool(name="projp", bufs=4, space="PSUM"))

            def add_bias(ps, boff, g):
                if has_bias:
                    nc.tensor.matmul(
                        ps[:],
                        ones_row[0:1, 0:128],
                        bias_sb[0:1, boff + g * 512: boff + g * 512 + 512],
                        start=False, stop=True,
                    )

            def elu1_half(dst_half, ps_half):
                # elu(x)+1 = exp(min(x,0)) + max(x,0), on a [128,512] half
                me = mepool.tile([128, 512], F32, tag="me")
                nc.vector.tensor_scalar_min(me[:], ps_half, 0.0)
                nc.scalar.activation(me[:], me[:], AF.Exp)
                nc.vector.scalar_tensor_tensor(
                    dst_half, ps_half, 0.0, me[:], ALU.max, ALU.add)

            # software pipeline over groups of GT=4 token tiles sharing one
            # x^T buffer:
            #   A1(t) = k/v projections for one tile (+ elu/copy on DVE)
            #   B(t)  = 2-head-batched kv matmuls for tile t (k^T @ [v|1])
            #   A2(g) = q^T computed directly (weights stationary, N=512)
            #           + elu in transposed layout, kept resident in SBUF
            # B(t-1) is emitted between A1 stages so the in-order PE always
            # has matmul work while DVE/ACT run elu.
            st = {}

            def stage_a1(t, xtg):
                tt = t % GT
                ksb = kqv_pool.tile([128, 1024], mm_dt, tag="k")
                vsb = kqv_pool.tile([128, 1040], mm_dt, tag="v")
                khalves = []
                for g in range(2):
                    kh = projp.tile([128, 512], F32, tag="proj", name=f"kps{t}_{g}")
                    for c in range(8):
                        nc.tensor.matmul(
                            kh[:], xtg[:, c * 512 + tt * 128: c * 512 + tt * 128 + 128],
                            wk_t[:, c * D + g * 512: c * D + g * 512 + 512],
                            start=(c == 0), stop=(c == 7 and not has_bias))
                    add_bias(kh, 1024, g)
                    khalves.append(kh)
                for g in range(2):
                    vh = projp.tile([128, 512], F32, tag="proj", name=f"vps{t}_{g}")
                    for c in range(8):
                        nc.tensor.matmul(
                            vh[:], xtg[:, c * 512 + tt * 128: c * 512 + tt * 128 + 128],
                            wv_t[:, c * D + g * 512: c * D + g * 512 + 512],
                            start=(c == 0), stop=(c == 7 and not has_bias))
                    add_bias(vh, 2048, g)
                    # strided copy into the [v | 1] augmented layout
                    nc.vector.tensor_copy(
                        vsb[:, g * 520: g * 520 + 520]
                        .rearrange("p (h e) -> p h e", e=65)[:, :, 0:64],
                        vh[:].rearrange("p (h e) -> p h e", e=64))
                nc.vector.memset(
                    vsb[:].rearrange("p (h e) -> p h e", e=65)[:, :, 64:65], 1.0)
                for g in range(2):
                    elu1_half(ksb[:, g * 512:(g + 1) * 512], khalves[g][:])
                st[t] = (ksb, vsb)

            def stage_b(t):
                ksb, vsb = st.pop(t)
                for r in range(8):
                    # NOTE: start=True clears has_written for the whole PSUM
                    # bank (2 blocks), so only the even block per bank sets it
                    nc.tensor.matmul(
                        kv_ps[:, r * 256: r * 256 + 130],
                        ksb[:, r * 128:(r + 1) * 128],
                        vsb[:, r * 130: r * 130 + 130],
                        start=(t == 0 and r % 2 == 0), stop=(t == NT - 1),
                    )

            def stage_a2(g, xtg):
                qtsb = qtpool.tile([128, 4096], mm_dt, tag=f"qt{g}")
                qts[g] = qtsb
                for dqc in range(8):
                    qh = projp.tile([128, 512], F32, tag="proj", name=f"qps{g}_{dqc}")
                    for dc in range(8):
                        nc.tensor.matmul(
                            qh[:],
                            wq_t[:, dc * D + dqc * 128: dc * D + dqc * 128 + 128],
                            xtg[:, dc * 512:(dc + 1) * 512],
                            start=(dc == 0), stop=(dc == 7 and not has_bias))
                    if has_bias:
                        # q^T bias: bq along partitions = rank-1 with ones row
                        nc.tensor.matmul(
                            qh[:],
                            bias_sb[0:1, dqc * 128: dqc * 128 + 128],
                            ones_row512[0:1, 0:512],
                            start=False, stop=True)
                    elu1_half(qtsb[:, dqc * 512:(dqc + 1) * 512], qh[:])

            def send_kv():
                # PSUM f32 -> bf16 in the PSUM-native block layout via the
                # scalar engine (the DVE queue is full of elu work), then
                # one DMA to the collective input
                ccsb = cpool.tile([128, 1040], mm_dt, tag="ccsb")
                nc.scalar.activation(
                    ccsb[:].rearrange("p (r w) -> p r w", w=130),
                    kv_ps[:].rearrange("p (r w) -> p r w", w=256)[:, :, 0:130],
                    AF.Copy)
                nc.sync.dma_start(cc_in[:], ccsb[:])
                nc.gpsimd.collective_compute(
                    "AllReduce", mybir.AluOpType.add,
                    replica_groups=[[0, 1], [2, 3], [4, 5], [6, 7]],
                    ins=[cc_in[:]], outs=[cc_out[:]],
                )

            prev = None
            xtgs = {}
            xdmas = {}
            for g in range(NG):
                xtg = xtpool.tile([128, GT * 1024], mm_dt, tag="xt")
                xtgs[g] = xtg
                if g == 0:
                    # per-chunk contiguous DMAs (1 KiB/partition runs) so the
                    # first matmuls only wait for chunk 0 and the loads keep
                    # pace with the c-loop consumption order
                    for c in range(8):
                        xdmas[g] = nc.sync.dma_start(
                            xtg[:, c * 512:(c + 1) * 512],
                            xst[:, c * 512:(c + 1) * 512])
                else:
                    xdmas[g] = nc.sync.dma_start(
                        xtg[:], xst[:, g * GT * 1024:(g + 1) * GT * 1024])
                if g == 1:
                    wq_t = load_weight(wqd, after=xdmas[1])
                if g == 3:
                    wo_t = load_weight(wod, after=xdmas[3])
                for tt in range(GT):
                    t = g * GT + tt
                    stage_a1(t, xtg)
                    if prev is not None:
                        stage_b(prev)
                    prev = t
            # finish kv, launch the AllReduce, THEN all q^T groups (~55us of
            # matmuls) hide the collective latency; the send chain (ACT copy
            # + one DMA + CC) runs concurrently since q^T touches neither
            # the sync queue nor kv data
            stage_b(prev)
            send_kv()
            for g in range(NG):
                stage_a2(g, xtgs[g])

        kvstack.close()

        # ---------------- Pass 2a: attention + normalize -> osb (SBUF) --------
        osbs = {}
        with ExitStack() as p2a:
            bdpool = p2a.enter_context(tc.tile_pool(name="bd", bufs=1))
            zpool = p2a.enter_context(tc.tile_pool(name="z", bufs=2))
            odp = p2a.enter_context(tc.tile_pool(name="odp", bufs=2, space="PSUM"))

            # block-diagonal [kv | ksum] matrix: chunk c (heads 2c, 2c+1):
            # rows 0:64 = head 2c (d), rows 64:128 = head 2c+1
            # cols c*256+[0:64] = kv_2c, [64:128] = kv_2c+1, 128/129 = ksums
            bd = bdpool.tile([128, 2048], mm_dt, tag="bd")
            nc.vector.memset(bd[:].bitcast(F32), 0.0)
            ccr_lo = cc_out[0:64, :].rearrange("p (c w) -> p c w", w=130)
            ccr_hi = cc_out[64:128, :].rearrange("p (c w) -> p c w", w=130)
            bd_lo = bd[0:64, :].rearrange("p (c r) -> p c r", r=256)
            bd_hi = bd[64:128, :].rearrange("p (c r) -> p c r", r=256)
            nc.sync.dma_start(bd_lo[:, :, 0:64], ccr_lo[:, :, 0:64])
            nc.sync.dma_start(bd_hi[:, :, 64:128], ccr_hi[:, :, 65:129])
            nc.sync.dma_start(bd_lo[:, :, 128:129], ccr_lo[:, :, 64:65])
            nc.sync.dma_start(bd_hi[:, :, 129:130], ccr_hi[:, :, 129:130])

            # attention for tile t; od double-buffered across tiles (all 8
            # PSUM banks) so attn(t+1) runs while DVE normalizes tile t
            for t in range(NT):
                g, tt = t // GT, t % GT
                qtsb = qts[g]
                ods = [odp.tile([128, 1024], F32, tag=f"od{i}", name=f"od{t}_{i}")
                       for i in range(2)]
                zden = zpool.tile([128, 16], F32, tag="zden")
                for half in range(2):
                    od = ods[half]
                    for cc in range(4):
                        c = half * 4 + cc
                        nc.tensor.matmul(
                            od[:, cc * 256:(cc + 1) * 256],
                            qtsb[:, c * 512 + tt * 128: c * 512 + tt * 128 + 128],
                            bd[:, c * 256:(c + 1) * 256],
                            start=True, stop=True,
                        )
                    od_r = od[:].rearrange("p (c r) -> p c r", r=256)
                    # scalar engine is idle here; keep the DVE free for the
                    # z-scale (gpsimd cannot access PSUM)
                    nc.scalar.activation(
                        zden[:, half * 8:(half + 1) * 8]
                        .rearrange("p (c i) -> p c i", i=2),
                        od_r[:, :, 128:130], AF.Copy)
                zinv = zpool.tile([128, 16], F32, tag="zinv")
                nc.vector.tensor_scalar_add(zden[:], zden[:], EPS)
                nc.vector.reciprocal(zinv[:], zden[:])
                osb = ospool.tile([128, 1024], mm_dt, tag=f"osb{t}")
                osbs[t] = osb
                for half in range(2):
                    od_r = ods[half][:].rearrange("p (c r) -> p c r", r=256)
                    zb = (zinv[:, half * 8:(half + 1) * 8]
                          .rearrange("p (c i) -> p c i", i=2)
                          .unsqueeze(3).broadcast_to((128, 4, 2, 64)))
                    nc.vector.tensor_mul(
                        osb[:, half * 512:(half + 1) * 512]
                        .rearrange("p (c i e) -> p c i e", c=4, i=2),
                        od_r[:, :, 0:128].rearrange("p c (i e) -> p c i e", i=2),
                        zb,
                    )

        # ---------------- Pass 2b: transpose + output projection --------------
        with ExitStack() as p2b:
            outT_pool = p2b.enter_context(tc.tile_pool(name="otb", bufs=2))
            y_pool = p2b.enter_context(tc.tile_pool(name="ysb", bufs=2))
            tpp2 = p2b.enter_context(tc.tile_pool(name="tpp2", bufs=2, space="PSUM"))
            ypp = p2b.enter_context(tc.tile_pool(name="ypp", bufs=2, space="PSUM"))

            for t in range(NT):
                osb = osbs.pop(t)
                otb = outT_pool.tile([128, 1024], mm_dt, tag="otb")
                for c in range(8):
                    tp2 = tpp2.tile([128, 128], mm_dt, tag="tp2")
                    nc.tensor.transpose(tp2[:], osb[:, c * 128:(c + 1) * 128],
                                        identm[:])
                    nc.vector.tensor_copy(otb[:, c * 128:(c + 1) * 128], tp2[:])

                yps = ypp.tile([128, 1024], F32, tag="y")
                for c in range(8):
                    lhs = otb[:, c * 128:(c + 1) * 128]
                    for g in range(2):
                        nc.tensor.matmul(
                            yps[:, g * 512:(g + 1) * 512], lhs,
                            wo_t[:, c * D + g * 512: c * D + g * 512 + 512],
                            start=(c == 0), stop=(c == 7),
                        )
                ysb = y_pool.tile([128, 1024], F32, tag="ysb")
                # scalar-engine copy keeps the DVE queue clear for the otb
                # copies the wo matmuls of the next tile wait on (gpsimd
                # cannot access PSUM)
                nc.scalar.activation(ysb[:], yps[:], AF.Copy)
                nc.sync.dma_start(y_d[t * 128:(t + 1) * 128, :], ysb[:])


def _get_program(has_bias):
    key = (has_bias, MM_DT)
    if key not in _PROGRAMS:
        nc = bacc.Bacc("TRN2", target_bir_lowering=False, debug=False,
                       num_devices=N_CORES)
        _emit(nc, has_bias, MM_DT)
        nc.compile()
        _PROGRAMS[key] = nc
    return _PROGRAMS[key]


def _to_mm_np(a):
    """Convert fp32 array to the numpy dtype matching MM_DT."""
    if MM_DT == BF16:
        import ml_dtypes
        return np.ascontiguousarray(a.astype(ml_dtypes.bfloat16))
    return np.ascontiguousarray(a)


def _pack_rhs(w):
    # W [out,in] -> W.T chunk-major rhs layout [128, 8*1024]:
    # [p, c*1024 + n] = W.T[c*128+p, n]
    return _to_mm_np(
        w.T.reshape(8, 128, D).transpose(1, 0, 2).reshape(128, 8 * D))


def _pack_xt(xs):
    # xs [TOK, D] -> x^T group-major: [p, g*4096 + c*512 + u] = xs[g*512+u, c*128+p]
    ng = NT // 4
    return _to_mm_np(
        xs.T.reshape(8, 128, ng, 512).transpose(1, 2, 0, 3).reshape(128, NT * 1024))


def kernel(x, Wq, bq, Wk, bk, Wv, bv, Wo, bo):
    global LAST_RESULT
    x = np.asarray(x, dtype=np.float32)
    Wq, Wk, Wv, Wo = (np.asarray(w, dtype=np.float32) for w in (Wq, Wk, Wv, Wo))
    bq, bk, bv, bo = (np.asarray(b, dtype=np.float32) for b in (bq, bk, bv, bo))

    has_bias = bool(np.any(bq) or np.any(bk) or np.any(bv))
    nc = _get_program(has_bias)
    shared = {
        "wkt": _pack_rhs(Wk),
        "wvt": _pack_rhs(Wv),
        "wqt": _pack_rhs(Wq),
        "wot": _pack_rhs(Wo),
        "ident": _to_mm_np(np.eye(128, dtype=np.float32)),
        "biases": np.concatenate([bq, bk, bv, bo]).reshape(1, 4096),
    }
    in_maps = []
    for c in range(N_CORES):
        b = c // 2
        h = c % 2
        m = dict(shared)
        m["xst"] = _pack_xt(x[b, h * TOK:(h + 1) * TOK, :])
        in_maps.append(m)

    res = run_bass_kernel_spmd(nc, in_maps, list(range(N_CORES)), trace=TRACE)
    LAST_RESULT = res

    y = np.empty((B, S, D), dtype=np.float32)
    for c in range(N_CORES):
        b = c // 2
        h = c % 2
        y[b, h * TOK:(h + 1) * TOK, :] = res.results[c]["y"]
    y += bo
    return y


# revision 25
# speedup vs baseline: 1.2248x; 1.0017x over previous
"""Linear self-attention (elu+1 feature map) Trainium2 kernel — bf16.

Problem: B=4, S=4096, D=1024, H=16, HD=64.
  q = elu1(x @ Wq.T + bq); k = elu1(x @ Wk.T + bk); v = x @ Wv.T + bv
  kv_h = k_h^T v_h; ksum_h = sum_t k_h; z = 1/(q.ksum + eps)
  out = (q_h @ kv_h) * z; y = out @ Wo.T + bo

Sharding: token-parallel. Core c handles batch c//2, sequence half c%2
(2048 tokens). kv/ksum are partial sums over local tokens, AllReduced
(bf16) across the 2-core group sharing a batch, then every core
finishes its own tokens through attention + output projection. bo is
added on host.

All PE-facing operands are bf16: fp32r moving operands run at ~2
cycles/row on TRN2 hardware and fp32 kv matmuls ran as LOW_HIGH pairs;
bf16 runs at 1 cycle/row. PSUM accumulation stays fp32. q^T stays
resident in SBUF (no DRAM spill). kv matmuls batch 2 heads per
instruction. Two q^T groups are deferred until after the kv AllReduce
send so the collective is hidden under matmul work.
"""

import numpy as np
from contextlib import ExitStack

import concourse.bass as bass
import concourse.tile as tile
from concourse import bacc, mybir
from concourse.bass_utils import run_bass_kernel_spmd
from concourse.tile_rust import add_dep_helper

B, S, D, H, HD = 4, 4096, 1024, 16, 64
N_CORES = 8
TOK = (B * S) // N_CORES      # 2048 tokens per core
NT = TOK // 128               # 16 token tiles per core
GT = 4                        # token tiles per x^T group
NG = NT // GT
F32 = mybir.dt.float32
BF16 = mybir.dt.bfloat16
EPS = 1e-6

MM_DT = BF16

TRACE = False            # set by test harness for profiling
LAST_RESULT = None       # BassKernelResults of last run

_PROGRAMS = {}


def _emit(nc, has_bias, mm_dt):
    AF = mybir.ActivationFunctionType
    ALU = mybir.AluOpType

    # x^T, chunk-major per token tile within each group:
    # [p, g*4096 + c*512 + u] = x[g*512 + u, c*128 + p]
    xst = nc.dram_tensor("xst", [128, NT * 1024], mm_dt, kind="ExternalInput").ap()
    wkd = nc.dram_tensor("wkt", [128, 8 * D], mm_dt, kind="ExternalInput").ap()
    wvd = nc.dram_tensor("wvt", [128, 8 * D], mm_dt, kind="ExternalInput").ap()
    wqd = nc.dram_tensor("wqt", [128, 8 * D], mm_dt, kind="ExternalInput").ap()
    wod = nc.dram_tensor("wot", [128, 8 * D], mm_dt, kind="ExternalInput").ap()
    identd = nc.dram_tensor("ident", [128, 128], mm_dt, kind="ExternalInput").ap()
    biasd = nc.dram_tensor("biases", [1, 4096], F32, kind="ExternalInput").ap()
    y_d = nc.dram_tensor("y", [TOK, D], F32, kind="ExternalOutput").ap()
    # kv collective payload keeps the PSUM block layout: block r (heads
    # 2r, 2r+1) at cols r*130; rows 0:64 x 0:65 = [kv_2r | ksum_2r],
    # rows 64:128 x 65:130 = [kv_2r+1 | ksum_2r+1] (complement is junk)
    cc_in = nc.dram_tensor("cc_in", [128, 1040], mm_dt).ap()
    cc_out = nc.dram_tensor("cc_out", [128, 1040], mm_dt).ap()

    with tile.TileContext(nc) as tc, ExitStack() as top:
        wpool = top.enter_context(tc.tile_pool(name="w", bufs=4))
        cpool = top.enter_context(tc.tile_pool(name="const", bufs=1))
        qtpool = top.enter_context(tc.tile_pool(name="qt", bufs=1))
        ospool = top.enter_context(tc.tile_pool(name="os", bufs=1))
        identm = cpool.tile([128, 128], mm_dt, tag="ident")
        nc.sync.dma_start(identm[:], identd)
        if has_bias:
            ones_row_st = cpool.tile([1, 512], F32, tag="ones_row_st")
            nc.vector.memset(ones_row_st[:], 1.0)
            ones_row = cpool.tile([1, 128], mm_dt, tag="ones_row")
            nc.vector.tensor_copy(ones_row[:], ones_row_st[0:1, 0:128])
            ones_row512 = cpool.tile([1, 512], mm_dt, tag="ones_row512")
            nc.vector.tensor_copy(ones_row512[:], ones_row_st[:])
            bias_st = cpool.tile([1, 3072], F32, tag="bias_st")
            nc.sync.dma_start(bias_st[:], biasd[0:1, 0:3072])
            bias_sb = cpool.tile([1, 3072], mm_dt, tag="bias")
            nc.vector.tensor_copy(bias_sb[:], bias_st[:])

        def load_weight(dram_ap, after=None):
            # weights stay off the sync queue (x tiles live there); chunked
            # so the first projection matmuls only wait for the first ~256 KiB.
            # `after` delays the load (sync dep on a prior instruction) so
            # late-use weights don't contend for early HBM bandwidth.
            wt = wpool.tile([128, 8 * D], mm_dt, tag="w")
            for c in range(8):
                dma = nc.gpsimd.dma_start(wt[:, c * D:(c + 1) * D],
                                          dram_ap[:, c * D:(c + 1) * D])
                if after is not None and c == 0:
                    add_dep_helper(dma.ins, after.ins, sync=True,
                                   reason="defer weight load off early HBM")
            return wt

        wk_t = load_weight(wkd)
        wv_t = load_weight(wvd)

        kvstack = ExitStack()
        kvpool = kvstack.enter_context(tc.tile_pool(name="kvp", bufs=1, space="PSUM"))
        # 2-head-batched kv accumulator: block r (heads 2r, 2r+1) at cols
        # r*256: rows 0:64 x cols 0:65 = [kv_2r | ksum_2r], rows 64:128 x
        # cols 65:130 = [kv_2r+1 | ksum_2r+1]; the other corners are unused
        kv_ps = kvpool.tile([128, 2048], F32, tag="kv")

        qts = {}

        # ---------------- Pass 1: q/k/v projections, kv + ksum ----------------
        with ExitStack() as p1:
            xtpool = p1.enter_context(tc.tile_pool(name="xt", bufs=4))
            kqv_pool = p1.enter_context(tc.tile_pool(name="kqv", bufs=2))
            mepool = p1.enter_context(tc.tile_pool(name="me", bufs=2))
            projp = p1.enter_context(tc.tile_pool(name="projp", bufs=4, space="PSUM"))

            def add_bias(ps, boff, g):
                if has_bias:
                    nc.tensor.matmul(
                        ps[:],
                        ones_row[0:1, 0:128],
                        bias_sb[0:1, boff + g * 512: boff + g * 512 + 512],
                        start=False, stop=True,
                    )

            def elu1_half(dst_half, ps_half):
                # elu(x)+1 = exp(min(x,0)) + max(x,0), on a [128,512] half
                me = mepool.tile([128, 512], F32, tag="me")
                nc.vector.tensor_scalar_min(me[:], ps_half, 0.0)
                nc.scalar.activation(me[:], me[:], AF.Exp)
                nc.vector.scalar_tensor_tensor(
                    dst_half, ps_half, 0.0, me[:], ALU.max, ALU.add)

            # software pipeline over groups of GT=4 token tiles sharing one
            # x^T buffer:
            #   A1(t) = k/v projections for one tile (+ elu/copy on DVE)
            #   B(t)  = 2-head-batched kv matmuls for tile t (k^T @ [v|1])
            #   A2(g) = q^T computed directly (weights stationary, N=512)
            #           + elu in transposed layout, kept resident in SBUF
            # B(t-1) is emitted between A1 stages so the in-order PE always
            # has matmul work while DVE/ACT run elu.
            st = {}

            def stage_a1(t, xtg):
                tt = t % GT
                ksb = kqv_pool.tile([128, 1024], mm_dt, tag="k")
                vsb = kqv_pool.tile([128, 1040], mm_dt, tag="v")
                khalves = []
                for g in range(2):
                    kh = projp.tile([128, 512], F32, tag="proj", name=f"kps{t}_{g}")
                    for c in range(8):
                        nc.tensor.matmul(
                            kh[:], xtg[:, c * 512 + tt * 128: c * 512 + tt * 128 + 128],
                            wk_t[:, c * D + g * 512: c * D + g * 512 + 512],
                            start=(c == 0), stop=(c == 7 and not has_bias))
                    add_bias(kh, 1024, g)
                    khalves.append(kh)
                for g in range(2):
                    vh = projp.tile([128, 512], F32, tag="proj", name=f"vps{t}_{g}")
                    for c in range(8):
                        nc.tensor.matmul(
                            vh[:], xtg[:, c * 512 + tt * 128: c * 512 + tt * 128 + 128],
                            wv_t[:, c * D + g * 512: c * D + g * 512 + 512],
                            start=(c == 0), stop=(c == 7 and not has_bias))
                    add_bias(vh, 2048, g)
                    # strided copy into the [v | 1] augmented layout
                    nc.vector.tensor_copy(
                        vsb[:, g * 520: g * 520 + 520]
                        .rearrange("p (h e) -> p h e", e=65)[:, :, 0:64],
                        vh[:].rearrange("p (h e) -> p h e", e=64))
                nc.vector.memset(
                    vsb[:].rearrange("p (h e) -> p h e", e=65)[:, :, 64:65], 1.0)
                for g in range(2):
                    elu1_half(ksb[:, g * 512:(g + 1) * 512], khalves[g][:])
                st[t] = (ksb, vsb)

            def stage_b(t):
                ksb, vsb = st.pop(t)
                for r in range(8):
                    # NOTE: start=True clears has_written for the whole PSUM
                    # bank (2 blocks), so only the even block per bank sets it
                    nc.tensor.matmul(
                        kv_ps[:, r * 256: r * 256 + 130],
                        ksb[:, r * 128:(r + 1) * 128],
                        vsb[:, r * 130: r * 130 + 130],
                        start=(t == 0 and r % 2 == 0), stop=(t == NT - 1),
                    )

            def stage_a2(g, xtg):
                qtsb = qtpool.tile([128, 4096], mm_dt, tag=f"qt{g}")
                qts[g] = qtsb
                for dqc in range(8):
                    qh = projp.tile([128, 512], F32, tag="proj", name=f"qps{g}_{dqc}")
                    for dc in range(8):
                        nc.tensor.matmul(
                            qh[:],
                            wq_t[:, dc * D + dqc * 128: dc * D + dqc * 128 + 128],
                            xtg[:, dc * 512:(dc + 1) * 512],
                            start=(dc == 0), stop=(dc == 7 and not has_bias))
                    if has_bias:
                        # q^T bias: bq along partitions = rank-1 with ones row
                        nc.tensor.matmul(
                            qh[:],
                            bias_sb[0:1, dqc * 128: dqc * 128 + 128],
                            ones_row512[0:1, 0:512],
                            start=False, stop=True)
                    elu1_half(qtsb[:, dqc * 512:(dqc + 1) * 512], qh[:])

            def send_kv():
                # PSUM f32 -> bf16 in the PSUM-native block layout via the
                # scalar engine (the DVE queue is full of elu work), then
                # one DMA to the collective input
                ccsb = cpool.tile([128, 1040], mm_dt, tag="ccsb")
                nc.scalar.activation(
                    ccsb[:].rearrange("p (r w) -> p r w", w=130),
                    kv_ps[:].rearrange("p (r w) -> p r w", w=256)[:, :, 0:130],
                    AF.Copy)
                nc.sync.dma_start(cc_in[:], ccsb[:])
                nc.gpsimd.collective_compute(
                    "AllReduce", mybir.AluOpType.add,
                    replica_groups=[[0, 1], [2, 3], [4, 5], [6, 7]],
                    ins=[cc_in[:]], outs=[cc_out[:]],
                )

            prev = None
            xtgs = {}
            xdmas = {}
            xprev = None
            for g in range(NG):
                xtg = xtpool.tile([128, GT * 1024], mm_dt, tag="xt")
                xtgs[g] = xtg
                if g == 0:
                    # per-chunk contiguous DMAs (1 KiB/partition runs),
                    # chained so chunks complete in the c-loop consumption
                    # order instead of all-at-once under HBM contention
                    for c in range(8):
                        dma = nc.sync.dma_start(
                            xtg[:, c * 512:(c + 1) * 512],
                            xst[:, c * 512:(c + 1) * 512])
                        if xprev is not None:
                            add_dep_helper(dma.ins, xprev.ins, sync=True,
                                           reason="x arrival order")
                        xprev = dma
                else:
                    dma = nc.sync.dma_start(
                        xtg[:], xst[:, g * GT * 1024:(g + 1) * GT * 1024])
                    add_dep_helper(dma.ins, xprev.ins, sync=True,
                                   reason="x arrival order")
                    xprev = dma
                xdmas[g] = xprev
                if g == 1:
                    wq_t = load_weight(wqd, after=xdmas[1])
                if g == 3:
                    wo_t = load_weight(wod, after=xdmas[3])
                for tt in range(GT):
                    t = g * GT + tt
                    stage_a1(t, xtg)
                    if prev is not None:
                        stage_b(prev)
                    prev = t
            # finish kv, launch the AllReduce, THEN all q^T groups (~55us of
            # matmuls) hide the collective latency; the send chain (ACT copy
            # + one DMA + CC) runs concurrently since q^T touches neither
            # the sync queue nor kv data
            stage_b(prev)
            send_kv()
            for g in range(NG):
                stage_a2(g, xtgs[g])

        kvstack.close()

        # ---------------- Pass 2a: attention + normalize -> osb (SBUF) --------
        osbs = {}
        with ExitStack() as p2a:
            bdpool = p2a.enter_context(tc.tile_pool(name="bd", bufs=1))
            zpool = p2a.enter_context(tc.tile_pool(name="z", bufs=2))
            odp = p2a.enter_context(tc.tile_pool(name="odp", bufs=2, space="PSUM"))

            # block-diagonal [kv | ksum] matrix: chunk c (heads 2c, 2c+1):
            # rows 0:64 = head 2c (d), rows 64:128 = head 2c+1
            # cols c*256+[0:64] = kv_2c, [64:128] = kv_2c+1, 128/129 = ksums
            bd = bdpool.tile([128, 2048], mm_dt, tag="bd")
            nc.vector.memset(bd[:].bitcast(F32), 0.0)
            ccr_lo = cc_out[0:64, :].rearrange("p (c w) -> p c w", w=130)
            ccr_hi = cc_out[64:128, :].rearrange("p (c w) -> p c w", w=130)
            bd_lo = bd[0:64, :].rearrange("p (c r) -> p c r", r=256)
            bd_hi = bd[64:128, :].rearrange("p (c r) -> p c r", r=256)
            nc.sync.dma_start(bd_lo[:, :, 0:64], ccr_lo[:, :, 0:64])
            nc.sync.dma_start(bd_hi[:, :, 64:128], ccr_hi[:, :, 65:129])
            nc.sync.dma_start(bd_lo[:, :, 128:129], ccr_lo[:, :, 64:65])
            nc.sync.dma_start(bd_hi[:, :, 129:130], ccr_hi[:, :, 129:130])

            # attention for tile t; od double-buffered across tiles (all 8
            # PSUM banks) so attn(t+1) runs while DVE normalizes tile t
            for t in range(NT):
                g, tt = t // GT, t % GT
                qtsb = qts[g]
                ods = [odp.tile([128, 1024], F32, tag=f"od{i}", name=f"od{t}_{i}")
                       for i in range(2)]
                zden = zpool.tile([128, 16], F32, tag="zden")
                for half in range(2):
                    od = ods[half]
                    for cc in range(4):
                        c = half * 4 + cc
                        nc.tensor.matmul(
                            od[:, cc * 256:(cc + 1) * 256],
                            qtsb[:, c * 512 + tt * 128: c * 512 + tt * 128 + 128],
                            bd[:, c * 256:(c + 1) * 256],
                            start=True, stop=True,
                        )
                    od_r = od[:].rearrange("p (c r) -> p c r", r=256)
                    # den + eps straight off PSUM on the idle scalar engine;
                    # keeps the DVE free for the z-scale
                    nc.scalar.activation(
                        zden[:, half * 8:(half + 1) * 8]
                        .rearrange("p (c i) -> p c i", i=2),
                        od_r[:, :, 128:130], AF.Copy, bias=EPS)
                zinv = zpool.tile([128, 16], F32, tag="zinv")
                nc.vector.reciprocal(zinv[:], zden[:])
                osb = ospool.tile([128, 1024], mm_dt, tag=f"osb{t}")
                osbs[t] = osb
                for half in range(2):
                    od_r = ods[half][:].rearrange("p (c r) -> p c r", r=256)
                    zb = (zinv[:, half * 8:(half + 1) * 8]
                          .rearrange("p (c i) -> p c i", i=2)
                          .unsqueeze(3).broadcast_to((128, 4, 2, 64)))
                    nc.vector.tensor_mul(
                        osb[:, half * 512:(half + 1) * 512]
                        .rearrange("p (c i e) -> p c i e", c=4, i=2),
                        od_r[:, :, 0:128].rearrange("p c (i e) -> p c i e", i=2),
                        zb,
                    )

        # ---------------- Pass 2b: transpose + output projection --------------
        with ExitStack() as p2b:
            outT_pool = p2b.enter_context(tc.tile_pool(name="otb", bufs=2))
            y_pool = p2b.enter_context(tc.tile_pool(name="ysb", bufs=2))
            tpp2 = p2b.enter_context(tc.tile_pool(name="tpp2", bufs=2, space="PSUM"))
            ypp = p2b.enter_context(tc.tile_pool(name="ypp", bufs=2, space="PSUM"))

            for t in range(NT):
                osb = osbs.pop(t)
                otb = outT_pool.tile([128, 1024], mm_dt, tag="otb")
                for c in range(8):
                    tp2 = tpp2.tile([128, 128], mm_dt, tag="tp2")
                    nc.tensor.transpose(tp2[:], osb[:, c * 128:(c + 1) * 128],
                                        identm[:])
                    nc.vector.tensor_copy(otb[:, c * 128:(c + 1) * 128], tp2[:])

                yps = ypp.tile([128, 1024], F32, tag="y")
                for c in range(8):
                    lhs = otb[:, c * 128:(c + 1) * 128]
                    for g in range(2):
                        nc.tensor.matmul(
                            yps[:, g * 512:(g + 1) * 512], lhs,
                            wo_t[:, c * D + g * 512: c * D + g * 512 + 512],
                            start=(c == 0), stop=(c == 7),
                        )
                ysb = y_pool.tile([128, 1024], F32, tag="ysb")
                # scalar-engine copy keeps the DVE queue clear for the otb
                # copies the wo matmuls of the next tile wait on (gpsimd
                # cannot access PSUM)
                nc.scalar.activation(ysb[:], yps[:], AF.Copy)
                nc.sync.dma_start(y_d[t * 128:(t + 1) * 128, :], ysb[:])


def _get_program(has_bias):
    key = (has_bias, MM_DT)
    if key not in _PROGRAMS:
        nc = bacc.Bacc("TRN2", target_bir_lowering=False, debug=False,
                       num_devices=N_CORES)
        _emit(nc, has_bias, MM_DT)
        nc.compile()
        _PROGRAMS[key] = nc
    return _PROGRAMS[key]


def _to_mm_np(a):
    """Convert fp32 array to the numpy dtype matching MM_DT."""
    if MM_DT == BF16:
        import ml_dtypes
        return np.ascontiguousarray(a.astype(ml_dtypes.bfloat16))
    return np.ascontiguousarray(a)


def _pack_rhs(w):
    # W [out,in] -> W.T chunk-major rhs layout [128, 8*1024]:
    # [p, c*1024 + n] = W.T[c*128+p, n]
    return _to_mm_np(
        w.T.reshape(8, 128, D).transpose(1, 0, 2).reshape(128, 8 * D))


def _pack_xt(xs):
    # xs [TOK, D] -> x^T group-major: [p, g*4096 + c*512 + u] = xs[g*512+u, c*128+p]
    ng = NT // 4
    return _to_mm_np(
        xs.T.reshape(8, 128, ng, 512).transpose(1, 2, 0, 3).reshape(128, NT * 1024))


def kernel(x, Wq, bq, Wk, bk, Wv, bv, Wo, bo):
    global LAST_RESULT
    x = np.asarray(x, dtype=np.float32)
    Wq, Wk, Wv, Wo = (np.asarray(w, dtype=np.float32) for w in (Wq, Wk, Wv, Wo))
    bq, bk, bv, bo = (np.asarray(b, dtype=np.float32) for b in (bq, bk, bv, bo))

    has_bias = bool(np.any(bq) or np.any(bk) or np.any(bv))
    nc = _get_program(has_bias)
    shared = {
        "wkt": _pack_rhs(Wk),
        "wvt": _pack_rhs(Wv),
        "wqt": _pack_rhs(Wq),
        "wot": _pack_rhs(Wo),
        "ident": _to_mm_np(np.eye(128, dtype=np.float32)),
        "biases": np.concatenate([bq, bk, bv, bo]).reshape(1, 4096),
    }
    in_maps = []
    for c in range(N_CORES):
        b = c // 2
        h = c % 2
        m = dict(shared)
        m["xst"] = _pack_xt(x[b, h * TOK:(h + 1) * TOK, :])
        in_maps.append(m)

    res = run_bass_kernel_spmd(nc, in_maps, list(range(N_CORES)), trace=TRACE)
    LAST_RESULT = res

    y = np.empty((B, S, D), dtype=np.float32)
    for c in range(N_CORES):
        b = c // 2
        h = c % 2
        y[b, h * TOK:(h + 1) * TOK, :] = res.results[c]["y"]
    y += bo
    return y


# revision 31
# speedup vs baseline: 1.2943x; 1.0567x over previous
"""Linear self-attention (elu+1 feature map) Trainium2 kernel — bf16.

Problem: B=4, S=4096, D=1024, H=16, HD=64.
  q = elu1(x @ Wq.T + bq); k = elu1(x @ Wk.T + bk); v = x @ Wv.T + bv
  kv_h = k_h^T v_h; ksum_h = sum_t k_h; z = 1/(q.ksum + eps)
  out = (q_h @ kv_h) * z; y = out @ Wo.T + bo

Sharding: token-parallel. Core c handles batch c//2, sequence half c%2
(2048 tokens). kv/ksum are partial sums over local tokens, AllReduced
(bf16) across the 2-core group sharing a batch, then every core
finishes its own tokens through attention + output projection. bo is
added on host.

All PE-facing operands are bf16: fp32r moving operands run at ~2
cycles/row on TRN2 hardware and fp32 kv matmuls ran as LOW_HIGH pairs;
bf16 runs at 1 cycle/row. PSUM accumulation stays fp32. q^T stays
resident in SBUF (no DRAM spill). kv matmuls batch 2 heads per
instruction. Two q^T groups are deferred until after the kv AllReduce
send so the collective is hidden under matmul work.
"""

import numpy as np
from contextlib import ExitStack

import concourse.bass as bass
import concourse.tile as tile
from concourse import bacc, mybir
from concourse.bass_utils import run_bass_kernel_spmd
from concourse.tile_rust import add_dep_helper

B, S, D, H, HD = 4, 4096, 1024, 16, 64
N_CORES = 8
TOK = (B * S) // N_CORES      # 2048 tokens per core
NT = TOK // 128               # 16 token tiles per core
GT = 4                        # token tiles per x^T group
NG = NT // GT
F32 = mybir.dt.float32
BF16 = mybir.dt.bfloat16
EPS = 1e-6

MM_DT = BF16

TRACE = False            # set by test harness for profiling
LAST_RESULT = None       # BassKernelResults of last run

_PROGRAMS = {}


def _emit(nc, has_bias, mm_dt):
    AF = mybir.ActivationFunctionType
    ALU = mybir.AluOpType

    # x^T, chunk-major per token tile within each group:
    # [p, g*4096 + c*512 + u] = x[g*512 + u, c*128 + p]
    xst = nc.dram_tensor("xst", [128, NT * 1024], mm_dt, kind="ExternalInput").ap()
    wkd = nc.dram_tensor("wkt", [128, 8 * D], mm_dt, kind="ExternalInput").ap()
    wvd = nc.dram_tensor("wvt", [128, 8 * D], mm_dt, kind="ExternalInput").ap()
    wqd = nc.dram_tensor("wqt", [128, 8 * D], mm_dt, kind="ExternalInput").ap()
    wod = nc.dram_tensor("wot", [128, 8 * D], mm_dt, kind="ExternalInput").ap()
    identd = nc.dram_tensor("ident", [128, 128], mm_dt, kind="ExternalInput").ap()
    biasd = nc.dram_tensor("biases", [1, 4096], F32, kind="ExternalInput").ap()
    # y ships bf16 (host upcasts); halves store traffic and the drain tail
    y_d = nc.dram_tensor("y", [TOK, D], mm_dt, kind="ExternalOutput").ap()
    # kv collective payload keeps the PSUM block layout: block r (heads
    # 2r, 2r+1) at cols r*130; rows 0:64 x 0:65 = [kv_2r | ksum_2r],
    # rows 64:128 x 65:130 = [kv_2r+1 | ksum_2r+1] (complement is junk)
    cc_in = nc.dram_tensor("cc_in", [128, 1040], mm_dt).ap()
    cc_out = nc.dram_tensor("cc_out", [128, 1040], mm_dt).ap()

    with tile.TileContext(nc) as tc, ExitStack() as top:
        wpool = top.enter_context(tc.tile_pool(name="w", bufs=4))
        cpool = top.enter_context(tc.tile_pool(name="const", bufs=1))
        qtpool = top.enter_context(tc.tile_pool(name="qt", bufs=1))
        ospool = top.enter_context(tc.tile_pool(name="os", bufs=1))
        identm = cpool.tile([128, 128], mm_dt, tag="ident")
        nc.scalar.dma_start(identm[:], identd)
        # block-diagonal [kv | ksum] matrix for pass 2 (chunk c = heads
        # 2c, 2c+1); zeroed now while the DVE is idle, filled after the CC
        bd = cpool.tile([128, 2048], mm_dt, tag="bd")
        nc.vector.memset(bd[:].bitcast(F32), 0.0)
        if has_bias:
            ones_row_st = cpool.tile([1, 512], F32, tag="ones_row_st")
            nc.vector.memset(ones_row_st[:], 1.0)
            ones_row = cpool.tile([1, 128], mm_dt, tag="ones_row")
            nc.vector.tensor_copy(ones_row[:], ones_row_st[0:1, 0:128])
            ones_row512 = cpool.tile([1, 512], mm_dt, tag="ones_row512")
            nc.vector.tensor_copy(ones_row512[:], ones_row_st[:])
            bias_st = cpool.tile([1, 3072], F32, tag="bias_st")
            nc.sync.dma_start(bias_st[:], biasd[0:1, 0:3072])
            bias_sb = cpool.tile([1, 3072], mm_dt, tag="bias")
            nc.vector.tensor_copy(bias_sb[:], bias_st[:])

        def load_weight(dram_ap, after=None):
            # weights stay off the sync queue (x tiles live there); chunked
            # so the first projection matmuls only wait for the first ~256 KiB.
            # `after` delays the load (sync dep on a prior instruction) so
            # late-use weights don't contend for early HBM bandwidth.
            wt = wpool.tile([128, 8 * D], mm_dt, tag="w")
            for c in range(8):
                dma = nc.gpsimd.dma_start(wt[:, c * D:(c + 1) * D],
                                          dram_ap[:, c * D:(c + 1) * D])
                if after is not None and c == 0:
                    add_dep_helper(dma.ins, after.ins, sync=True,
                                   reason="defer weight load off early HBM")
            return wt

        wk_t = load_weight(wkd)
        wv_t = load_weight(wvd)

        kvstack = ExitStack()
        kvpool = kvstack.enter_context(tc.tile_pool(name="kvp", bufs=1, space="PSUM"))
        # 2-head-batched kv accumulator: block r (heads 2r, 2r+1) at cols
        # r*256: rows 0:64 x cols 0:65 = [kv_2r | ksum_2r], rows 64:128 x
        # cols 65:130 = [kv_2r+1 | ksum_2r+1]; the other corners are unused
        kv_ps = kvpool.tile([128, 2048], F32, tag="kv")

        qts = {}

        # ---------------- Pass 1: q/k/v projections, kv + ksum ----------------
        with ExitStack() as p1:
            xtpool = p1.enter_context(tc.tile_pool(name="xt", bufs=4))
            kqv_pool = p1.enter_context(tc.tile_pool(name="kqv", bufs=2))
            mepool = p1.enter_context(tc.tile_pool(name="me", bufs=2))
            projp = p1.enter_context(tc.tile_pool(name="projp", bufs=4, space="PSUM"))

            def add_bias(ps, boff, g):
                if has_bias:
                    nc.tensor.matmul(
                        ps[:],
                        ones_row[0:1, 0:128],
                        bias_sb[0:1, boff + g * 512: boff + g * 512 + 512],
                        start=False, stop=True,
                    )

            def elu1_half(dst_half, ps_half):
                # elu(x)+1 = exp(min(x,0)) + max(x,0), on a [128,512] half
                me = mepool.tile([128, 512], F32, tag="me")
                nc.vector.tensor_scalar_min(me[:], ps_half, 0.0)
                nc.scalar.activation(me[:], me[:], AF.Exp)
                nc.vector.scalar_tensor_tensor(
                    dst_half, ps_half, 0.0, me[:], ALU.max, ALU.add)

            # software pipeline over groups of GT=4 token tiles sharing one
            # x^T buffer:
            #   A1(t) = k/v projections for one tile (+ elu/copy on DVE)
            #   B(t)  = 2-head-batched kv matmuls for tile t (k^T @ [v|1])
            #   A2(g) = q^T computed directly (weights stationary, N=512)
            #           + elu in transposed layout, kept resident in SBUF
            # B(t-1) is emitted between A1 stages so the in-order PE always
            # has matmul work while DVE/ACT run elu.
            st = {}

            def stage_a1(t, xtg):
                tt = t % GT
                ksb = kqv_pool.tile([128, 1024], mm_dt, tag="k")
                vsb = kqv_pool.tile([128, 1040], mm_dt, tag="v")
                khalves = []
                for g in range(2):
                    kh = projp.tile([128, 512], F32, tag="proj", name=f"kps{t}_{g}")
                    for c in range(8):
                        nc.tensor.matmul(
                            kh[:], xtg[:, c * 512 + tt * 128: c * 512 + tt * 128 + 128],
                            wk_t[:, c * D + g * 512: c * D + g * 512 + 512],
                            start=(c == 0), stop=(c == 7 and not has_bias))
                    add_bias(kh, 1024, g)
                    khalves.append(kh)
                for g in range(2):
                    vh = projp.tile([128, 512], F32, tag="proj", name=f"vps{t}_{g}")
                    for c in range(8):
                        nc.tensor.matmul(
                            vh[:], xtg[:, c * 512 + tt * 128: c * 512 + tt * 128 + 128],
                            wv_t[:, c * D + g * 512: c * D + g * 512 + 512],
                            start=(c == 0), stop=(c == 7 and not has_bias))
                    add_bias(vh, 2048, g)
                    # strided copy into the [v | 1] augmented layout
                    nc.vector.tensor_copy(
                        vsb[:, g * 520: g * 520 + 520]
                        .rearrange("p (h e) -> p h e", e=65)[:, :, 0:64],
                        vh[:].rearrange("p (h e) -> p h e", e=64))
                nc.vector.memset(
                    vsb[:].rearrange("p (h e) -> p h e", e=65)[:, :, 64:65], 1.0)
                for g in range(2):
                    elu1_half(ksb[:, g * 512:(g + 1) * 512], khalves[g][:])
                st[t] = (ksb, vsb)

            def stage_b(t):
                ksb, vsb = st.pop(t)
                for r in range(8):
                    # NOTE: start=True clears has_written for the whole PSUM
                    # bank (2 blocks), so only the even block per bank sets it
                    nc.tensor.matmul(
                        kv_ps[:, r * 256: r * 256 + 130],
                        ksb[:, r * 128:(r + 1) * 128],
                        vsb[:, r * 130: r * 130 + 130],
                        start=(t == 0 and r % 2 == 0), stop=(t == NT - 1),
                    )

            def stage_a2(g, xtg):
                qtsb = qtpool.tile([128, 4096], mm_dt, tag=f"qt{g}")
                qts[g] = qtsb
                for dqc in range(8):
                    qh = projp.tile([128, 512], F32, tag="proj", name=f"qps{g}_{dqc}")
                    for dc in range(8):
                        nc.tensor.matmul(
                            qh[:],
                            wq_t[:, dc * D + dqc * 128: dc * D + dqc * 128 + 128],
                            xtg[:, dc * 512:(dc + 1) * 512],
                            start=(dc == 0), stop=(dc == 7 and not has_bias))
                    if has_bias:
                        # q^T bias: bq along partitions = rank-1 with ones row
                        nc.tensor.matmul(
                            qh[:],
                            bias_sb[0:1, dqc * 128: dqc * 128 + 128],
                            ones_row512[0:1, 0:512],
                            start=False, stop=True)
                    elu1_half(qtsb[:, dqc * 512:(dqc + 1) * 512], qh[:])

            def send_kv():
                # PSUM f32 -> bf16 in the PSUM-native block layout via the
                # scalar engine (the DVE queue is full of elu work), then
                # one DMA to the collective input
                ccsb = cpool.tile([128, 1040], mm_dt, tag="ccsb")
                nc.scalar.activation(
                    ccsb[:].rearrange("p (r w) -> p r w", w=130),
                    kv_ps[:].rearrange("p (r w) -> p r w", w=256)[:, :, 0:130],
                    AF.Copy)
                nc.sync.dma_start(cc_in[:], ccsb[:])
                nc.gpsimd.collective_compute(
                    "AllReduce", mybir.AluOpType.add,
                    replica_groups=[[0, 1], [2, 3], [4, 5], [6, 7]],
                    ins=[cc_in[:]], outs=[cc_out[:]],
                )

            prev = None
            xtgs = {}
            xdmas = {}
            xprev = None
            for g in range(NG):
                xtg = xtpool.tile([128, GT * 1024], mm_dt, tag="xt")
                xtgs[g] = xtg
                if g == 0:
                    # 8 concurrent per-chunk DMAs (one engine each) so group
                    # 0 gets a large share of HBM against the weight loads
                    for c in range(8):
                        xprev = nc.sync.dma_start(
                            xtg[:, c * 512:(c + 1) * 512],
                            xst[:, c * 512:(c + 1) * 512])
                else:
                    # 4 concurrent quarter DMAs, staggered to start after the
                    # previous group finished so they don't steal engines
                    # from data needed sooner
                    first = None
                    for q4 in range(4):
                        lo = g * GT * 1024 + q4 * 1024
                        dma = nc.sync.dma_start(
                            xtg[:, q4 * 1024:(q4 + 1) * 1024],
                            xst[:, lo:lo + 1024])
                        if q4 == 0:
                            add_dep_helper(dma.ins, xprev.ins, sync=True,
                                           reason="stagger x group loads")
                        xprev = dma
                xdmas[g] = xprev
                if g == 1:
                    wq_t = load_weight(wqd, after=xdmas[1])
                if g == 3:
                    wo_t = load_weight(wod, after=xdmas[3])
                for tt in range(GT):
                    t = g * GT + tt
                    stage_a1(t, xtg)
                    if prev is not None:
                        stage_b(prev)
                    prev = t
            # finish kv, launch the AllReduce, THEN all q^T groups (~55us of
            # matmuls) hide the collective latency; the send chain (ACT copy
            # + one DMA + CC) runs concurrently since q^T touches neither
            # the sync queue nor kv data
            stage_b(prev)
            send_kv()
            for g in range(NG):
                stage_a2(g, xtgs[g])

        kvstack.close()

        # ---------------- Pass 2a: attention + normalize -> osb (SBUF) --------
        osbs = {}
        with ExitStack() as p2a:
            zpool = p2a.enter_context(tc.tile_pool(name="z", bufs=2))
            odp = p2a.enter_context(tc.tile_pool(name="odp", bufs=2, space="PSUM"))

            # fill bd: rows 0:64 = head 2c (d), rows 64:128 = head 2c+1
            # cols c*256+[0:64] = kv_2c, [64:128] = kv_2c+1, 128/129 = ksums
            ccr_lo = cc_out[0:64, :].rearrange("p (c w) -> p c w", w=130)
            ccr_hi = cc_out[64:128, :].rearrange("p (c w) -> p c w", w=130)
            bd_lo = bd[0:64, :].rearrange("p (c r) -> p c r", r=256)
            bd_hi = bd[64:128, :].rearrange("p (c r) -> p c r", r=256)
            nc.sync.dma_start(bd_lo[:, :, 0:64], ccr_lo[:, :, 0:64])
            nc.sync.dma_start(bd_hi[:, :, 64:128], ccr_hi[:, :, 65:129])
            nc.sync.dma_start(bd_lo[:, :, 128:129], ccr_lo[:, :, 64:65])
            nc.sync.dma_start(bd_hi[:, :, 129:130], ccr_hi[:, :, 129:130])

            # attention for tile t; od double-buffered across tiles (all 8
            # PSUM banks) so attn(t+1) runs while DVE normalizes tile t
            for t in range(NT):
                g, tt = t // GT, t % GT
                qtsb = qts[g]
                ods = [odp.tile([128, 1024], F32, tag=f"od{i}", name=f"od{t}_{i}")
                       for i in range(2)]
                zden = zpool.tile([128, 16], F32, tag="zden")
                for half in range(2):
                    od = ods[half]
                    for cc in range(4):
                        c = half * 4 + cc
                        nc.tensor.matmul(
                            od[:, cc * 256:(cc + 1) * 256],
                            qtsb[:, c * 512 + tt * 128: c * 512 + tt * 128 + 128],
                            bd[:, c * 256:(c + 1) * 256],
                            start=True, stop=True,
                        )
                    od_r = od[:].rearrange("p (c r) -> p c r", r=256)
                    # den + eps straight off PSUM on the idle scalar engine;
                    # keeps the DVE free for the z-scale
                    nc.scalar.activation(
                        zden[:, half * 8:(half + 1) * 8]
                        .rearrange("p (c i) -> p c i", i=2),
                        od_r[:, :, 128:130], AF.Copy, bias=EPS)
                zinv = zpool.tile([128, 16], F32, tag="zinv")
                nc.vector.reciprocal(zinv[:], zden[:])
                osb = ospool.tile([128, 1024], mm_dt, tag=f"osb{t}")
                osbs[t] = osb
                for half in range(2):
                    od_r = ods[half][:].rearrange("p (c r) -> p c r", r=256)
                    zb = (zinv[:, half * 8:(half + 1) * 8]
                          .rearrange("p (c i) -> p c i", i=2)
                          .unsqueeze(3).broadcast_to((128, 4, 2, 64)))
                    nc.vector.tensor_mul(
                        osb[:, half * 512:(half + 1) * 512]
                        .rearrange("p (c i e) -> p c i e", c=4, i=2),
                        od_r[:, :, 0:128].rearrange("p c (i e) -> p c i e", i=2),
                        zb,
                    )

        # ---------------- Pass 2b: transpose + output projection --------------
        with ExitStack() as p2b:
            outT_pool = p2b.enter_context(tc.tile_pool(name="otb", bufs=2))
            y_pool = p2b.enter_context(tc.tile_pool(name="ysb", bufs=2))
            tpp2 = p2b.enter_context(tc.tile_pool(name="tpp2", bufs=2, space="PSUM"))
            ypp = p2b.enter_context(tc.tile_pool(name="ypp", bufs=2, space="PSUM"))

            for t in range(NT):
                osb = osbs.pop(t)
                otb = outT_pool.tile([128, 1024], mm_dt, tag="otb")
                for c in range(8):
                    tp2 = tpp2.tile([128, 128], mm_dt, tag="tp2")
                    nc.tensor.transpose(tp2[:], osb[:, c * 128:(c + 1) * 128],
                                        identm[:])
                    nc.vector.tensor_copy(otb[:, c * 128:(c + 1) * 128], tp2[:])

                yps = ypp.tile([128, 1024], F32, tag="y")
                for c in range(8):
                    lhs = otb[:, c * 128:(c + 1) * 128]
                    for g in range(2):
                        nc.tensor.matmul(
                            yps[:, g * 512:(g + 1) * 512], lhs,
                            wo_t[:, c * D + g * 512: c * D + g * 512 + 512],
                            start=(c == 0), stop=(c == 7),
                        )
                ysb = y_pool.tile([128, 1024], mm_dt, tag="ysb")
                # scalar-engine copy keeps the DVE queue clear for the otb
                # copies the wo matmuls of the next tile wait on (gpsimd
                # cannot access PSUM)
                nc.scalar.activation(ysb[:], yps[:], AF.Copy)
                nc.sync.dma_start(y_d[t * 128:(t + 1) * 128, :], ysb[:])


def _get_program(has_bias):
    key = (has_bias, MM_DT)
    if key not in _PROGRAMS:
        nc = bacc.Bacc("TRN2", target_bir_lowering=False, debug=False,
                       num_devices=N_CORES)
        _emit(nc, has_bias, MM_DT)
        nc.compile()
        _PROGRAMS[key] = nc
    return _PROGRAMS[key]


def _to_mm_np(a):
    """Convert fp32 array to the numpy dtype matching MM_DT."""
    if MM_DT == BF16:
        import ml_dtypes
        return np.ascontiguousarray(a.astype(ml_dtypes.bfloat16))
    return np.ascontiguousarray(a)


def _pack_rhs(w):
    # W [out,in] -> W.T chunk-major rhs layout [128, 8*1024]:
    # [p, c*1024 + n] = W.T[c*128+p, n]
    return _to_mm_np(
        w.T.reshape(8, 128, D).transpose(1, 0, 2).reshape(128, 8 * D))


def _pack_xt(xs):
    # xs [TOK, D] -> x^T group-major: [p, g*4096 + c*512 + u] = xs[g*512+u, c*128+p]
    ng = NT // 4
    return _to_mm_np(
        xs.T.reshape(8, 128, ng, 512).transpose(1, 2, 0, 3).reshape(128, NT * 1024))


def kernel(x, Wq, bq, Wk, bk, Wv, bv, Wo, bo):
    global LAST_RESULT
    x = np.asarray(x, dtype=np.float32)
    Wq, Wk, Wv, Wo = (np.asarray(w, dtype=np.float32) for w in (Wq, Wk, Wv, Wo))
    bq, bk, bv, bo = (np.asarray(b, dtype=np.float32) for b in (bq, bk, bv, bo))

    has_bias = bool(np.any(bq) or np.any(bk) or np.any(bv))
    nc = _get_program(has_bias)
    shared = {
        "wkt": _pack_rhs(Wk),
        "wvt": _pack_rhs(Wv),
        "wqt": _pack_rhs(Wq),
        "wot": _pack_rhs(Wo),
        "ident": _to_mm_np(np.eye(128, dtype=np.float32)),
        "biases": np.concatenate([bq, bk, bv, bo]).reshape(1, 4096),
    }
    in_maps = []
    for c in range(N_CORES):
        b = c // 2
        h = c % 2
        m = dict(shared)
        m["xst"] = _pack_xt(x[b, h * TOK:(h + 1) * TOK, :])
        in_maps.append(m)

    res = run_bass_kernel_spmd(nc, in_maps, list(range(N_CORES)), trace=TRACE)
    LAST_RESULT = res

    y = np.empty((B, S, D), dtype=np.float32)
    for c in range(N_CORES):
        b = c // 2
        h = c % 2
        y[b, h * TOK:(h + 1) * TOK, :] = np.asarray(
            res.results[c]["y"]).astype(np.float32)
    y += bo
    return y
